# revision 29
# baseline (speedup 1.0000x reference)
"""Trainium2 Bass kernel for nn_DetectionLoss (B=16, M=8, H=W=112, C=64, N=20).

Strategy (pure data parallel over batch, 2 images per core on 8 cores):

  - The objectness BCE negative term sum(ln(1-p)) over all M*H*W cells is
    streamed through the Scalar engine (Ln activation with fused free-axis
    accumulation) — the only full-tensor pass.
  - Everything else touches only the <=20 GT cells per image.  The HW
    indirect-DMA gather semantics here are: ONE offset per destination
    partition row, reading a PHYSICALLY CONTIGUOUS run from the source
    (src-view strides are ignored; `coef` = product of src dims after the
    offset axis scales the offset).  So the host PRE-PACKS a per-cell
    tensor pack[i, s, 0:552] = [obj scores (m) | boxes (m,k) | classes
    (m,c)] and a single indirect DMA with host-known offsets (i*HW+s)
    lands each GT's full working set in one SBUF partition row.
  - Slot selection (first m with score>0.5 else 0) and the slot-dependent
    selection of boxes/logits run on device via is_equal masks against
    host-provided m-grids, so no second (device-offset) gather round-trip
    is needed.
  - GIoU runs on the Pool engine, focal CE glue on DVE, exp/ln on Scalar,
    all overlapping; one fused TensorE matmul against 0/1 indicator
    columns produces all per-image sums in a single [4,4] PSUM tile.
  - Host work is limited to integer/index/layout prep (transposes of the
    input tensors, one-hots from gt_labels, cell indices from gt_boxes)
    and the final 16->3 reduction; all floating-point loss math over the
    input values runs on device.

Sync-wait discipline (this walrus build encodes at most 1 wait on compute
instructions, 2 on DMA): per engine, the first consumer of each DMA is
ordered so every instruction adds at most one new semaphore wait.
"""
import sys

if "/opt/trn_rl_repo" not in sys.path:
    sys.path.insert(0, "/opt/trn_rl_repo")

import numpy as np

B, M, H, W, C, N = 16, 8, 112, 112, 64, 20
NCORES = 8
BC = B // NCORES          # images per core
NN = BC * N               # gt rows per core
HW = H * W                # 12544
OBJ_TOT = BC * M * HW     # 200704 = 128 * 1568
FREE = OBJ_TOT // 128     # 1568
NT = 2                    # column tiles for the objectness stream
FW = FREE // NT

PK = 8 + M * 4 + M * C    # 552 pack columns per cell
PACK_TOT = BC * HW * PK

POS_W = 10.0
ALPHA = 0.25
EPS = 1e-7
OBJ_W, BOX_W, CLS_W = 0.1, 1.0, 1.0

HOT = 31                  # hot cols: pidx|gt4|oh_t|valid|alpha|m1000|mgrid8
COLD = 68                 # cold: ohc64|ind4

_PROG = None


def _install_drain_patch():
    """This walrus build only encodes a limited number of sync waits on the
    CTRL (drain) instruction; Tile's end-of-kernel drain can exceed it.
    Split the waits across a chain of single-wait SP nops instead."""
    import concourse.tile as tile_mod
    import concourse.mybir as mb
    from concourse.vector_clock import ScopedClock

    if getattr(tile_mod.TileContext, "_drain_patch_installed", False):
        return

    def _patched(self, tick_clock, wait_clock):
        nc = self.nc
        probe = nc.engines[mb.EngineType.SP].nop()
        wait_clock.add_sem_waits(
            probe.ins, ScopedClock({None: tick_clock.global_clock})
        )
        si = probe.ins.sync_info
        waits = list(si.on_wait) if (si is not None and si.on_wait) else []
        if len(waits) > 1:
            probe.ins.sync_info = mb.SyncInfo(
                on_wait=[waits[0]], on_update=si.on_update
            )
            for w in waits[1:]:
                extra = nc.engines[mb.EngineType.SP].nop()
                extra.ins.sync_info = mb.SyncInfo(on_wait=[w], on_update=[])
        nc.sync.drain()

        nc.all_engine_barrier()
        assert self.sems is not None
        popped = nc._tile_sem_poison_stack.pop()
        assert popped is self._sem_poison
        nc.clear_and_free_semaphores(list(self.sems.allocated().values()))
        nc.all_engine_barrier()

    tile_mod.TileContext._drain_and_barrier = _patched
    tile_mod.TileContext._drain_patch_installed = True


def build_program():
    import concourse.bass as bass
    import concourse.mybir as mybir
    import concourse.tile as tile

    _install_drain_patch()
    dt = mybir.dt
    AF = mybir.ActivationFunctionType
    OP = mybir.AluOpType
    AX = mybir.AxisListType.X

    nc = bass.Bass()
    f32, i32 = dt.float32, dt.int32
    obj = nc.declare_dram_parameter("obj", [OBJ_TOT], f32, isOutput=False)
    pack = nc.declare_dram_parameter("pack", [PACK_TOT], f32, isOutput=False)
    ph = nc.declare_dram_parameter("ph", [NN, HOT], f32, isOutput=False)
    pcold = nc.declare_dram_parameter("pc", [128, COLD], f32, isOutput=False)
    osum = nc.declare_dram_parameter("osum", [4, 4], f32, isOutput=True)

    IOff = bass.IndirectOffsetOnAxis
    packv = pack.rearrange("(x c) -> x c", c=PK)       # coef = PK on axis 0
    objv = obj.rearrange("(p f) -> p f", p=128)

    with tile.TileContext(nc) as tc:
        with (
            tc.tile_pool(name="sb", bufs=1) as sb,
            tc.tile_pool(name="ps", bufs=1, space="PSUM") as ps,
        ):
            # ---------------- t0: DMAs, memsets, act-table preload --------
            # issues spread across engine sequencers so the transfers land
            # on parallel queues and hot (the gather's dep) goes first
            # hot goes through Pool's own queue so the gather (also Pool)
            # sees it with minimal cross-queue latency
            t_ph = sb.tile([NN, HOT], f32)
            nc.gpsimd.dma_start(t_ph[:], ph[:])
            t_pc = sb.tile([128, COLD], f32)
            nc.scalar.dma_start(t_pc[:], pcold[:])
            t_str = [sb.tile([128, FW], f32, name=f"t_str{t}")
                     for t in range(NT)]
            nc.sync.dma_start(t_str[0][:], objv[:, 0:FW])
            nc.scalar.dma_start(t_str[1][:], objv[:, FW:2 * FW])

            t_R = sb.tile([128, 4], f32)
            nc.vector.memset(t_R[:], 0.0)
            t_dmy = sb.tile([1, 1], f32)
            nc.gpsimd.memset(t_dmy[:], 0.0)
            t_dmy2 = sb.tile([1, 1], f32)
            # early dummy activation: forces the (single) Ln/Exp act-table
            # load to overlap the input DMAs instead of the critical path
            nc.scalar.activation(t_dmy2[:], t_dmy[:], AF.Exp)

            # hot param views
            t_gt = t_ph[:, 1:5]
            t_oht = t_ph[:, 5:13]
            t_va = t_ph[:, 13:14]
            t_al = t_ph[:, 14:15]
            t_m1000 = t_ph[:, 15:23]
            t_mg8 = t_ph[:, 23:31]
            # cold param views
            t_ohc = t_pc[0:NN, 0:64]
            t_ind = t_pc[:, 64:68]

            # ---------------- the one gather (Pool) -----------------------
            t_pack = sb.tile([NN, PK], f32)
            nc.gpsimd.indirect_dma_start(
                t_pack[:], None, packv,
                IOff(ap=t_ph[:, 0:1].bitcast(i32), axis=0),
            )
            t_sc = t_pack[:, 0:8]
            t_bx = t_pack[:, 8:40]
            t_cl = t_pack[:, 40:PK]

            # ---------------- objectness stream (Scal) --------------------
            # separate accum tiles: a shared one would add a same-engine
            # WAW semaphore wait on top of the chunk-DMA wait (cap 1)
            t_acc0 = sb.tile([128, 1], f32)
            t_acc1 = sb.tile([128, 1], f32)
            t_staccs = [t_acc0, t_acc1]
            t_strouts = [sb.tile([128, FW], f32, name=f"t_strout{t}")
                         for t in range(NT)]
            for t in range(NT):
                nc.scalar.activation(
                    t_strouts[t][:], t_str[t][:], AF.Ln, scale=-1.0, bias=1.0,
                    accum_out=t_staccs[t][:],
                )

            # ---------------- slot chain (DVE) ----------------------------
            # T8 cols: [p_cx p_cy p_w p_h | t_cx t_cy t_w t_h]
            T8 = sb.tile([NN, 8], f32)
            t_sel = sb.tile([NN, M], f32)
            nc.vector.tensor_single_scalar(t_sel[:], t_sc, 0.5, OP.is_gt)
            nc.vector.tensor_copy(T8[:, 4:8], t_gt)   # observes hot DMA
            t_v = sb.tile([NN, M], f32)
            nc.vector.scalar_tensor_tensor(
                t_v[:], t_sel[:], -1000.0, t_m1000, OP.mult, OP.add)
            t_ft = sb.tile([NN, 1], f32)
            nc.vector.tensor_reduce(t_ft[:], t_v[:], AX, OP.min)
            t_any = sb.tile([NN, 1], f32)
            nc.vector.tensor_single_scalar(t_any[:], t_ft[:], 900.0, OP.is_lt)
            t_slot = sb.tile([NN, 1], f32)
            nc.vector.tensor_tensor(t_slot[:], t_ft[:], t_any[:], OP.mult)
            # ppos = scores . onehot(slot_t)  (head of the positive-cell
            # correction; the Pool-side product doubles as Pool's observer
            # of the gather DMA, the tiny reduce runs on DVE)
            t_ppj = sb.tile([NN, M], f32)
            nc.gpsimd.tensor_tensor(t_ppj[:], t_sc, t_oht, OP.mult)
            t_pp = sb.tile([NN, 1], f32)
            nc.vector.tensor_reduce(t_pp[:], t_ppj[:], AX, OP.add)

            # -------- positive-cell BCE correction tail (Pool+Scal) -------
            # corr = -10*ln(max(p,eps)) + ln(max(1-p,eps))
            t_L2 = sb.tile([NN, 2], f32)
            nc.gpsimd.tensor_single_scalar(
                t_L2[:, 0:1], t_pp[:], 1e-38, OP.max)
            t_1p = sb.tile([NN, 1], f32)
            nc.gpsimd.tensor_scalar(
                t_1p[:], t_pp[:], -1.0, 1.0, OP.mult, OP.add)
            nc.gpsimd.tensor_single_scalar(
                t_L2[:, 1:2], t_1p[:], 1e-38, OP.max)
            t_L2l = sb.tile([NN, 2], f32)
            nc.scalar.activation(t_L2l[:], t_L2[:], AF.Ln)
            t_L2c = sb.tile([NN, 2], f32)
            nc.gpsimd.tensor_single_scalar(t_L2c[:], t_L2l[:], -100.0, OP.max)
            t_l10 = sb.tile([NN, 1], f32)
            nc.gpsimd.tensor_scalar_mul(t_l10[:], t_L2c[:, 0:1], -POS_W)
            t_co = sb.tile([NN, 1], f32)
            nc.gpsimd.tensor_tensor(t_co[:], t_l10[:], t_L2c[:, 1:2], OP.add)

            # ---------------- class logits at slot (DVE) ------------------
            # indD doubles as the DVE cold-DMA observer (before xjunk)
            t_indD = sb.tile([128, 4], f32)
            nc.vector.tensor_copy(t_indD[:], t_ind)
            # slot one-hot over m, broadcast over c / k via stride-0 views
            t_oh8 = sb.tile([NN, M], f32)
            bm0, bm1 = bass.broadcast_tensor_aps(t_mg8, t_slot[:])
            nc.vector.tensor_tensor(t_oh8[:], bm0, bm1, OP.is_equal)
            a8 = t_oh8[:]
            oh8_c = bass.AP(a8.tensor, a8.offset,
                            [list(a8.ap[0]), [0, C], list(a8.ap[1])])
            oh8_k = bass.AP(a8.tensor, a8.offset,
                            [list(a8.ap[0]), list(a8.ap[1]), [0, 4]])
            t_m512 = sb.tile([NN, M * C], f32)
            nc.vector.tensor_tensor(
                t_m512[:].rearrange("p (c m) -> p c m", m=M),
                t_cl.rearrange("p (c m) -> p c m", m=M), oh8_c, OP.mult)
            # pack classes are (c, m) so the m-reduction is contiguous
            t_log64 = sb.tile([NN, C], f32)
            nc.vector.tensor_reduce(
                t_log64[:], t_m512[:].rearrange("p (c m) -> p c m", m=M),
                AX, OP.add)
            # box selection (same one-hot, broadcast over k)
            t_m32 = sb.tile([NN, 32], f32)
            nc.vector.tensor_tensor(
                t_m32[:].rearrange("p (m k) -> p m k", k=4),
                t_bx.rearrange("p (m k) -> p m k", k=4), oh8_k, OP.mult)

            # focal CE — ce/pt/om/sq chained on Scalar (AP bias) to avoid
            # cross-engine ping-pong; xl in parallel on DVE
            t_exp = sb.tile([NN, C], f32)
            t_se = sb.tile([NN, 1], f32)
            nc.scalar.activation(t_exp[:], t_log64[:], AF.Exp,
                                 accum_out=t_se[:])
            t_lse = sb.tile([NN, 1], f32)
            nc.scalar.activation(t_lse[:], t_se[:], AF.Ln)
            t_xjunk = sb.tile([NN, C], f32)
            nc.vector.tensor_tensor(t_xjunk[:], t_log64[:], t_ohc, OP.mult)
            t_xl = sb.tile([NN, 1], f32)
            nc.vector.tensor_reduce(t_xl[:], t_xjunk[:], AX, OP.add)
            t_lsec = sb.tile([NN, 1], f32)
            nc.vector.tensor_copy(t_lsec[:], t_lse[:])   # Act observer
            t_ce = sb.tile([NN, 1], f32)
            nc.vector.tensor_tensor(t_ce[:], t_lsec[:], t_xl[:], OP.subtract)
            t_pt = sb.tile([NN, 1], f32)
            nc.scalar.activation(t_pt[:], t_ce[:], AF.Exp, scale=-1.0)
            t_om = sb.tile([NN, 1], f32)
            nc.scalar.activation(t_om[:], t_pt[:], AF.Copy, scale=-1.0,
                                 bias=1.0 - EPS)
            t_sq = sb.tile([NN, 1], f32)
            nc.scalar.activation(t_sq[:], t_om[:], AF.Square)
            t_cal = sb.tile([NN, 1], f32)
            nc.vector.tensor_tensor(t_cal[:], t_ce[:], t_al, OP.mult)
            nc.vector.tensor_tensor(t_R[0:NN, 1:2], t_sq[:], t_cal[:], OP.mult)
            # stream sums -> R col 3 (act2 tick already observed above)
            nc.vector.tensor_tensor(
                t_R[:, 3:4], t_acc0[:], t_acc1[:], OP.add)

            # ---------------- GIoU (Pool, bx4/recip on DVE) ---------------
            nc.vector.tensor_reduce(
                T8[:, 0:4], t_m32[:].rearrange("p (m k) -> p k m", k=4),
                AX, OP.add)

            # Pool assembles Q = [lo_p lo_t | hi_p hi_t] and the pa/ta
            # products; DVE does the min/max pairs and the divide chain
            # (overlapping the Scalar focal chain).
            T8v = T8[:].rearrange("p (b k) -> p b k", k=4)
            t_wh2 = sb.tile([NN, 4], f32)
            t_wh2v = t_wh2[:].rearrange("p (b k) -> p b k", k=2)
            nc.gpsimd.tensor_scalar_mul(t_wh2v, T8v[:, :, 2:4], 0.5)
            t_pt2 = sb.tile([NN, 2], f32)    # [pa, ta]
            nc.gpsimd.tensor_tensor(
                t_pt2[:].rearrange("p (b o) -> p b o", o=1),
                T8v[:, :, 2:3], T8v[:, :, 3:4], OP.mult)
            t_s1 = sb.tile([NN, 1], f32)
            nc.gpsimd.tensor_tensor(t_s1[:], t_pt2[:, 0:1], t_pt2[:, 1:2],
                                    OP.add)
            # Q after s1, so X1's single [Pool>=Qhi] wait covers s1 too
            t_Q = sb.tile([NN, 8], f32)
            nc.gpsimd.tensor_tensor(
                t_Q[:, 0:4].rearrange("p (b k) -> p b k", k=2),
                T8v[:, :, 0:2], t_wh2v, OP.subtract)
            nc.gpsimd.tensor_tensor(
                t_Q[:, 4:8].rearrange("p (b k) -> p b k", k=2),
                T8v[:, :, 0:2], t_wh2v, OP.add)

            # DVE: X1 = [i1 | e2], X2 = [e1 | i2]  (min/max is DVE-only)
            Qh = t_Q[:].rearrange("p (h x) -> p h x", h=2)
            t_X1 = sb.tile([NN, 4], f32)
            nc.vector.tensor_tensor(
                t_X1[:].rearrange("p (h k) -> p h k", k=2),
                Qh[:, :, 0:2], Qh[:, :, 2:4], OP.max)
            t_X2 = sb.tile([NN, 4], f32)
            nc.vector.tensor_tensor(
                t_X2[:].rearrange("p (h k) -> p h k", k=2),
                Qh[:, :, 0:2], Qh[:, :, 2:4], OP.min)
            # W2 = [iwc_x iwc_y ew_x ew_y]
            t_iw = sb.tile([NN, 2], f32)
            nc.vector.tensor_tensor(t_iw[:], t_X2[:, 2:4], t_X1[:, 0:2],
                                    OP.subtract)
            t_W2 = sb.tile([NN, 4], f32)
            nc.vector.tensor_single_scalar(t_W2[:, 0:2], t_iw[:], 0.0, OP.max)
            nc.vector.tensor_tensor(t_W2[:, 2:4], t_X1[:, 2:4], t_X2[:, 0:2],
                                    OP.subtract)
            # ie = [inter, enc]
            t_ie = sb.tile([NN, 2], f32)
            W2v = t_W2[:].rearrange("p (x y) -> p x y", y=2)
            nc.vector.tensor_tensor(
                t_ie[:].rearrange("p (x o) -> p x o", o=1),
                W2v[:, :, 0:1], W2v[:, :, 1:2], OP.mult)
            t_d2 = sb.tile([NN, 2], f32)     # [union, enc]
            nc.vector.tensor_tensor(t_d2[:, 0:1], t_s1[:], t_ie[:, 0:1],
                                    OP.subtract)
            nc.vector.tensor_copy(t_d2[:, 1:2], t_ie[:, 1:2])
            t_d2a = sb.tile([NN, 2], f32)
            nc.vector.tensor_single_scalar(t_d2a[:], t_d2[:], 1e-6, OP.add)
            t_r2 = sb.tile([NN, 2], f32)
            nc.vector.reciprocal(t_r2[:], t_d2a[:])
            t_iou = sb.tile([NN, 1], f32)
            nc.vector.tensor_tensor(t_iou[:], t_ie[:, 0:1], t_r2[:, 0:1],
                                    OP.mult)
            t_em = sb.tile([NN, 1], f32)
            nc.vector.tensor_tensor(t_em[:], t_ie[:, 1:2], t_d2[:, 0:1],
                                    OP.subtract)
            t_q = sb.tile([NN, 1], f32)
            nc.vector.tensor_tensor(t_q[:], t_em[:], t_r2[:, 1:2], OP.mult)
            t_gi = sb.tile([NN, 1], f32)
            nc.vector.tensor_tensor(t_gi[:], t_iou[:], t_q[:], OP.subtract)

            # ---------------- R finalization (DVE only) & writeback -------
            # tm = clip(1 - clip(gi,-1,1), 0) == clip(1-gi, 0, 2)
            t_h1 = sb.tile([NN, 1], f32)
            nc.vector.tensor_scalar(t_h1[:], t_gi[:], -1.0, 1.0, OP.mult,
                                    OP.add)
            nc.vector.tensor_scalar(t_R[0:NN, 0:1], t_h1[:], 0.0, 2.0,
                                    OP.max, OP.min)
            nc.vector.tensor_tensor(t_R[0:NN, 2:3], t_co[:], t_va, OP.mult)
            ps_out = ps.tile([4, 4], f32)
            nc.tensor.matmul(ps_out[:], t_R[:], t_indD[:])
            t_os = sb.tile([4, 4], f32)
            nc.vector.tensor_copy(t_os[:], ps_out[:])
            nc.sync.dma_start(osum[:], t_os[:])

    nc.finalize()
    for blk in nc.m.functions[0].blocks:
        for ins in blk.instructions:
            si = ins.sync_info
            nw = len(si.on_wait) if (si and si.on_wait) else 0
            cap = 2 if type(ins).__name__ == "InstDMACopy" else 1
            if nw > cap:
                import os as _os
                if _os.environ.get("BASSDL_NO_WAIT_ASSERT"):
                    print("WAITVIOLATION", type(ins).__name__, ins.name,
                          ins.engine, [x.ant_name for x in si.on_wait])
                else:
                    raise AssertionError(
                        f"{type(ins).__name__} {ins.name} has {nw} sync waits "
                        f"(cap {cap} in this walrus build) — restructure deps")
    return nc


def host_prep(objectness, boxes, classes, gt_boxes, gt_labels):
    """Build the 8 per-core input maps.  Index/one-hot prep from gt_* plus
    pure layout transforms (transposes) of the float inputs — no float
    loss math happens here."""
    objectness = np.ascontiguousarray(np.asarray(objectness, dtype=np.float32))
    boxes = np.asarray(boxes, dtype=np.float32)
    classes = np.asarray(classes, dtype=np.float32)
    gb = np.asarray(gt_boxes, dtype=np.float32)
    gl = np.asarray(gt_labels).astype(np.int64)

    cx = np.clip((gb[:, :, 0] * np.float32(W)).astype(np.int32), 0, W - 1)
    cy = np.clip((gb[:, :, 1] * np.float32(H)).astype(np.int32), 0, H - 1)
    s = (cy * W + cx).astype(np.int64)                      # [B,N]
    eq = s[:, :, None] == s[:, None, :]                     # [B,N,N]
    tril = np.tril(np.ones((N, N), dtype=bool), k=-1)
    rank = (eq & tril[None]).sum(axis=2)                    # [B,N]
    valid = rank < M
    slot_t = np.minimum(rank, M - 1)

    # cold params
    cold = np.zeros((128, COLD), np.float32)
    for i in range(BC):
        cold[N * i:N * (i + 1), 64 + i] = 1.0               # ind20
        cold[64 * i:64 * (i + 1), 66 + i] = -1.0            # ind_neg

    in_maps = []
    for c in range(NCORES):
        bsel = slice(BC * c, BC * (c + 1))
        sB = s[bsel]                                        # [BC,N]
        il = np.arange(BC, dtype=np.int64)[:, None]
        pidx = (il * HW + sB).reshape(NN).astype(np.int32)

        glc = gl[bsel].reshape(NN)
        ohc = np.zeros((NN, C), np.float32)
        ohc[np.arange(NN), glc] = 1.0
        al = np.where(glc == 0, np.float32(ALPHA), np.float32(1 - ALPHA))
        va = valid[bsel].reshape(NN).astype(np.float32)
        oht = np.zeros((NN, M), np.float32)
        oht[np.arange(NN), slot_t[bsel].reshape(NN)] = 1.0

        hot = np.zeros((NN, HOT), np.float32)
        hot[:, 0] = pidx.view(np.float32)
        hot[:, 1:5] = gb[bsel].reshape(NN, 4)
        hot[:, 5:13] = oht
        hot[:, 13] = va
        hot[:, 14] = al
        hot[:, 15:23] = (np.arange(M) + 1000.0).astype(np.float32)[None, :]
        hot[:, 23:31] = np.arange(M, dtype=np.float32)[None, :]

        coldc = cold.copy()
        coldc[0:NN, 0:64] = ohc

        pk = np.empty((BC, HW, PK), np.float32)
        pk[:, :, 0:8] = objectness[bsel].transpose(0, 2, 3, 1).reshape(
            BC, HW, M)
        pk[:, :, 8:40] = boxes[bsel].transpose(0, 3, 4, 1, 2).reshape(
            BC, HW, M * 4)
        pk[:, :, 40:PK] = classes[bsel].transpose(0, 3, 4, 2, 1).reshape(
            BC, HW, C * M)

        in_maps.append({
            "obj": objectness[bsel].reshape(-1),
            "pack": pk.reshape(-1),
            "ph": hot,
            "pc": coldc,
        })
    return in_maps


def assemble(results):
    """Unshard: per-core [4,4] sums -> three weighted scalar means."""
    box, cls_, objl = [], [], []
    for r in results:
        o = np.asarray(r["osum"], dtype=np.float32)
        for i in range(BC):
            box.append(o[0, i] / np.float32(N))
            cls_.append(o[1, i] / np.float32(N))
            objl.append((o[2, i] + o[3, 2 + i]) / np.float32(M * HW))
    bl = np.float32(np.sum(np.asarray(box, np.float32)) / np.float32(B))
    cl = np.float32(np.sum(np.asarray(cls_, np.float32)) / np.float32(B))
    ol = np.float32(np.sum(np.asarray(objl, np.float32)) / np.float32(B))
    return (np.float32(bl * np.float32(BOX_W)),
            np.float32(cl * np.float32(CLS_W)),
            np.float32(ol * np.float32(OBJ_W)))


def _get_program():
    global _PROG
    if _PROG is None:
        _PROG = build_program()
    return _PROG


LAST_RESULTS = None  # BassKernelResults of the most recent run (for test.py)


def kernel(objectness, boxes, classes, gt_boxes, gt_labels):
    import os
    from concourse.bass_utils import run_bass_kernel_spmd

    global LAST_RESULTS
    nc = _get_program()
    in_maps = host_prep(objectness, boxes, classes, gt_boxes, gt_labels)
    trace = bool(os.environ.get("BASSDL_TRACE"))
    res = run_bass_kernel_spmd(nc, in_maps, list(range(NCORES)), trace=trace)
    LAST_RESULTS = res
    return assemble(res.results)


# revision 30
# speedup vs baseline: 1.1771x; 1.1771x over previous
"""Trainium2 Bass kernel for nn_DetectionLoss (B=16, M=8, H=W=112, C=64, N=20).

Strategy (pure data parallel over batch, 2 images per core on 8 cores):

  - The objectness BCE negative term sum(ln(1-p)) over all M*H*W cells is
    streamed through the Scalar engine (Ln activation with fused free-axis
    accumulation) — the only full-tensor pass.
  - Everything else touches only the <=20 GT cells per image.  The HW
    indirect-DMA gather semantics here are: ONE offset per destination
    partition row, reading a PHYSICALLY CONTIGUOUS run from the source
    (src-view strides are ignored; `coef` = product of src dims after the
    offset axis scales the offset).  So the host PRE-PACKS a per-cell
    tensor pack[i, s, 0:552] = [obj scores (m) | boxes (m,k) | classes
    (m,c)] and a single indirect DMA with host-known offsets (i*HW+s)
    lands each GT's full working set in one SBUF partition row.
  - Slot selection (first m with score>0.5 else 0) and the slot-dependent
    selection of boxes/logits run on device via is_equal masks against
    host-provided m-grids, so no second (device-offset) gather round-trip
    is needed.
  - GIoU runs on the Pool engine, focal CE glue on DVE, exp/ln on Scalar,
    all overlapping; one fused TensorE matmul against 0/1 indicator
    columns produces all per-image sums in a single [4,4] PSUM tile.
  - Host work is limited to integer/index/layout prep (transposes of the
    input tensors, one-hots from gt_labels, cell indices from gt_boxes)
    and the final 16->3 reduction; all floating-point loss math over the
    input values runs on device.

Sync-wait discipline (this walrus build encodes at most 1 wait on compute
instructions, 2 on DMA): per engine, the first consumer of each DMA is
ordered so every instruction adds at most one new semaphore wait.
"""
import sys

if "/opt/trn_rl_repo" not in sys.path:
    sys.path.insert(0, "/opt/trn_rl_repo")

import numpy as np

B, M, H, W, C, N = 16, 8, 112, 112, 64, 20
NCORES = 8
BC = B // NCORES          # images per core
NN = BC * N               # gt rows per core
HW = H * W                # 12544
OBJ_TOT = BC * M * HW     # 200704 = 128 * 1568
FREE = OBJ_TOT // 128     # 1568
NT = 2                    # column tiles for the objectness stream
FW = FREE // NT

PK = 8 + M * 4 + M * C    # 552 pack columns per cell
PACK_TOT = BC * HW * PK

POS_W = 10.0
ALPHA = 0.25
EPS = 1e-7
OBJ_W, BOX_W, CLS_W = 0.1, 1.0, 1.0

HOT = 31                  # hot cols: pidx|gt4|oh_t|valid|alpha|m1000|mgrid8
COLD = 68                 # cold: ohc64|ind4

_PROG = None


def _install_drain_patch():
    """This walrus build only encodes a limited number of sync waits on the
    CTRL (drain) instruction; Tile's end-of-kernel drain can exceed it.
    Split the waits across a chain of single-wait SP nops instead."""
    import concourse.tile as tile_mod
    import concourse.mybir as mb
    from concourse.vector_clock import ScopedClock

    if getattr(tile_mod.TileContext, "_drain_patch_installed", False):
        return

    def _patched(self, tick_clock, wait_clock):
        nc = self.nc
        probe = nc.engines[mb.EngineType.SP].nop()
        wait_clock.add_sem_waits(
            probe.ins, ScopedClock({None: tick_clock.global_clock})
        )
        si = probe.ins.sync_info
        waits = list(si.on_wait) if (si is not None and si.on_wait) else []
        if len(waits) > 1:
            probe.ins.sync_info = mb.SyncInfo(
                on_wait=[waits[0]], on_update=si.on_update
            )
            for w in waits[1:]:
                extra = nc.engines[mb.EngineType.SP].nop()
                extra.ins.sync_info = mb.SyncInfo(on_wait=[w], on_update=[])
        nc.sync.drain()

        nc.all_engine_barrier()
        assert self.sems is not None
        popped = nc._tile_sem_poison_stack.pop()
        assert popped is self._sem_poison
        nc.clear_and_free_semaphores(list(self.sems.allocated().values()))
        nc.all_engine_barrier()

    tile_mod.TileContext._drain_and_barrier = _patched
    tile_mod.TileContext._drain_patch_installed = True


def build_program():
    import concourse.bass as bass
    import concourse.mybir as mybir
    import concourse.tile as tile

    _install_drain_patch()
    dt = mybir.dt
    AF = mybir.ActivationFunctionType
    OP = mybir.AluOpType
    AX = mybir.AxisListType.X

    nc = bass.Bass()
    f32, i32 = dt.float32, dt.int32
    obj = nc.declare_dram_parameter("obj", [OBJ_TOT], f32, isOutput=False)
    pack = nc.declare_dram_parameter("pack", [PACK_TOT], f32, isOutput=False)
    ph = nc.declare_dram_parameter("ph", [NN, HOT], f32, isOutput=False)
    pcold = nc.declare_dram_parameter("pc", [128, COLD], f32, isOutput=False)
    osum = nc.declare_dram_parameter("osum", [4, 4], f32, isOutput=True)

    IOff = bass.IndirectOffsetOnAxis
    packv = pack.rearrange("(x c) -> x c", c=PK)       # coef = PK on axis 0
    objv = obj.rearrange("(p f) -> p f", p=128)

    with tile.TileContext(nc) as tc:
        with (
            tc.tile_pool(name="sb", bufs=1) as sb,
            tc.tile_pool(name="ps", bufs=1, space="PSUM") as ps,
        ):
            # ---------------- t0: DMAs, memsets, act-table preload --------
            # issues spread across engine sequencers so the transfers land
            # on parallel queues and hot (the gather's dep) goes first
            # hot issues first and alone on SP so its queue drains
            # immediately; the big stream DMAs are issued from Pool AFTER
            # the gather instruction so their transfers cannot crowd the
            # gather's packets out of the DMA engines
            t_ph = sb.tile([NN, HOT], f32)
            nc.sync.dma_start(t_ph[:], ph[:])
            t_pc = sb.tile([128, COLD], f32)
            nc.scalar.dma_start(t_pc[:], pcold[:])
            t_str = [sb.tile([128, FW], f32, name=f"t_str{t}")
                     for t in range(NT)]

            t_R = sb.tile([128, 4], f32)
            nc.vector.memset(t_R[:], 0.0)
            t_dmy = sb.tile([1, 1], f32)
            nc.gpsimd.memset(t_dmy[:], 0.0)
            t_dmy2 = sb.tile([1, 1], f32)
            # early dummy activation: forces the (single) Ln/Exp act-table
            # load to overlap the input DMAs instead of the critical path
            nc.scalar.activation(t_dmy2[:], t_dmy[:], AF.Exp)

            # hot param views
            t_gt = t_ph[:, 1:5]
            t_oht = t_ph[:, 5:13]
            t_va = t_ph[:, 13:14]
            t_al = t_ph[:, 14:15]
            t_m1000 = t_ph[:, 15:23]
            t_mg8 = t_ph[:, 23:31]
            # cold param views
            t_ohc = t_pc[0:NN, 0:64]
            t_ind = t_pc[:, 64:68]

            # ---------------- the one gather (Pool) -----------------------
            t_pack = sb.tile([NN, PK], f32)
            nc.gpsimd.indirect_dma_start(
                t_pack[:], None, packv,
                IOff(ap=t_ph[:, 0:1].bitcast(i32), axis=0),
            )
            t_sc = t_pack[:, 0:8]
            t_bx = t_pack[:, 8:40]
            t_cl = t_pack[:, 40:PK]
            # stream DMAs issued behind the gather on Pool (see above)
            nc.gpsimd.dma_start(t_str[0][:], objv[:, 0:FW])
            nc.gpsimd.dma_start(t_str[1][:], objv[:, FW:2 * FW])

            # ---------------- objectness stream (Scal) --------------------
            # separate accum tiles: a shared one would add a same-engine
            # WAW semaphore wait on top of the chunk-DMA wait (cap 1)
            t_acc0 = sb.tile([128, 1], f32)
            t_acc1 = sb.tile([128, 1], f32)
            t_staccs = [t_acc0, t_acc1]
            t_strouts = [sb.tile([128, FW], f32, name=f"t_strout{t}")
                         for t in range(NT)]
            for t in range(NT):
                nc.scalar.activation(
                    t_strouts[t][:], t_str[t][:], AF.Ln, scale=-1.0, bias=1.0,
                    accum_out=t_staccs[t][:],
                )

            # ---------------- slot chain (DVE) ----------------------------
            # T8 cols: [p_cx p_cy p_w p_h | t_cx t_cy t_w t_h]
            T8 = sb.tile([NN, 8], f32)
            t_sel = sb.tile([NN, M], f32)
            nc.vector.tensor_single_scalar(t_sel[:], t_sc, 0.5, OP.is_gt)
            nc.vector.tensor_copy(T8[:, 4:8], t_gt)   # observes hot DMA
            t_v = sb.tile([NN, M], f32)
            nc.vector.scalar_tensor_tensor(
                t_v[:], t_sel[:], -1000.0, t_m1000, OP.mult, OP.add)
            t_ft = sb.tile([NN, 1], f32)
            nc.vector.tensor_reduce(t_ft[:], t_v[:], AX, OP.min)
            t_any = sb.tile([NN, 1], f32)
            nc.vector.tensor_single_scalar(t_any[:], t_ft[:], 900.0, OP.is_lt)
            t_slot = sb.tile([NN, 1], f32)
            nc.vector.tensor_tensor(t_slot[:], t_ft[:], t_any[:], OP.mult)
            # ppos = scores . onehot(slot_t)  (head of the positive-cell
            # correction; the Pool-side product doubles as Pool's observer
            # of the gather DMA, the tiny reduce runs on DVE)
            t_ppj = sb.tile([NN, M], f32)
            nc.gpsimd.tensor_tensor(t_ppj[:], t_sc, t_oht, OP.mult)
            t_pp = sb.tile([NN, 1], f32)
            nc.vector.tensor_reduce(t_pp[:], t_ppj[:], AX, OP.add)

            # -------- positive-cell BCE correction tail (Pool+Scal) -------
            # corr = -10*ln(max(p,eps)) + ln(max(1-p,eps))
            t_L2 = sb.tile([NN, 2], f32)
            nc.gpsimd.tensor_single_scalar(
                t_L2[:, 0:1], t_pp[:], 1e-38, OP.max)
            t_1p = sb.tile([NN, 1], f32)
            nc.gpsimd.tensor_scalar(
                t_1p[:], t_pp[:], -1.0, 1.0, OP.mult, OP.add)
            nc.gpsimd.tensor_single_scalar(
                t_L2[:, 1:2], t_1p[:], 1e-38, OP.max)
            t_L2l = sb.tile([NN, 2], f32)
            nc.scalar.activation(t_L2l[:], t_L2[:], AF.Ln)
            t_L2c = sb.tile([NN, 2], f32)
            nc.gpsimd.tensor_single_scalar(t_L2c[:], t_L2l[:], -100.0, OP.max)
            t_l10 = sb.tile([NN, 1], f32)
            nc.gpsimd.tensor_scalar_mul(t_l10[:], t_L2c[:, 0:1], -POS_W)
            t_co = sb.tile([NN, 1], f32)
            nc.gpsimd.tensor_tensor(t_co[:], t_l10[:], t_L2c[:, 1:2], OP.add)

            # ---------------- class logits at slot (DVE) ------------------
            # indD doubles as the DVE cold-DMA observer (before xjunk)
            t_indD = sb.tile([128, 4], f32)
            nc.vector.tensor_copy(t_indD[:], t_ind)
            # slot one-hot over m, broadcast over c / k via stride-0 views
            t_oh8 = sb.tile([NN, M], f32)
            bm0, bm1 = bass.broadcast_tensor_aps(t_mg8, t_slot[:])
            nc.vector.tensor_tensor(t_oh8[:], bm0, bm1, OP.is_equal)
            a8 = t_oh8[:]
            oh8_c = bass.AP(a8.tensor, a8.offset,
                            [list(a8.ap[0]), [0, C], list(a8.ap[1])])
            oh8_k = bass.AP(a8.tensor, a8.offset,
                            [list(a8.ap[0]), list(a8.ap[1]), [0, 4]])
            t_m512 = sb.tile([NN, M * C], f32)
            nc.vector.tensor_tensor(
                t_m512[:].rearrange("p (c m) -> p c m", m=M),
                t_cl.rearrange("p (c m) -> p c m", m=M), oh8_c, OP.mult)
            # pack classes are (c, m) so the m-reduction is contiguous
            t_log64 = sb.tile([NN, C], f32)
            nc.vector.tensor_reduce(
                t_log64[:], t_m512[:].rearrange("p (c m) -> p c m", m=M),
                AX, OP.add)
            # box selection (same one-hot, broadcast over k)
            t_m32 = sb.tile([NN, 32], f32)
            nc.vector.tensor_tensor(
                t_m32[:].rearrange("p (m k) -> p m k", k=4),
                t_bx.rearrange("p (m k) -> p m k", k=4), oh8_k, OP.mult)

            # focal CE — ce/pt/om/sq chained on Scalar (AP bias) to avoid
            # cross-engine ping-pong; xl in parallel on DVE
            t_exp = sb.tile([NN, C], f32)
            t_se = sb.tile([NN, 1], f32)
            nc.scalar.activation(t_exp[:], t_log64[:], AF.Exp,
                                 accum_out=t_se[:])
            t_lse = sb.tile([NN, 1], f32)
            nc.scalar.activation(t_lse[:], t_se[:], AF.Ln)
            t_xjunk = sb.tile([NN, C], f32)
            nc.vector.tensor_tensor(t_xjunk[:], t_log64[:], t_ohc, OP.mult)
            t_xl = sb.tile([NN, 1], f32)
            nc.vector.tensor_reduce(t_xl[:], t_xjunk[:], AX, OP.add)
            t_lsec = sb.tile([NN, 1], f32)
            nc.vector.tensor_copy(t_lsec[:], t_lse[:])   # Act observer
            t_ce = sb.tile([NN, 1], f32)
            nc.vector.tensor_tensor(t_ce[:], t_lsec[:], t_xl[:], OP.subtract)
            t_pt = sb.tile([NN, 1], f32)
            nc.scalar.activation(t_pt[:], t_ce[:], AF.Exp, scale=-1.0)
            t_om = sb.tile([NN, 1], f32)
            nc.scalar.activation(t_om[:], t_pt[:], AF.Copy, scale=-1.0,
                                 bias=1.0 - EPS)
            t_sq = sb.tile([NN, 1], f32)
            nc.scalar.activation(t_sq[:], t_om[:], AF.Square)
            t_cal = sb.tile([NN, 1], f32)
            nc.vector.tensor_tensor(t_cal[:], t_ce[:], t_al, OP.mult)
            nc.vector.tensor_tensor(t_R[0:NN, 1:2], t_sq[:], t_cal[:], OP.mult)
            # stream sums -> R col 3 (act2 tick already observed above)
            nc.vector.tensor_tensor(
                t_R[:, 3:4], t_acc0[:], t_acc1[:], OP.add)

            # ---------------- GIoU (Pool, bx4/recip on DVE) ---------------
            nc.vector.tensor_reduce(
                T8[:, 0:4], t_m32[:].rearrange("p (m k) -> p k m", k=4),
                AX, OP.add)

            # Pool assembles Q = [lo_p lo_t | hi_p hi_t] and the pa/ta
            # products; DVE does the min/max pairs and the divide chain
            # (overlapping the Scalar focal chain).
            T8v = T8[:].rearrange("p (b k) -> p b k", k=4)
            t_wh2 = sb.tile([NN, 4], f32)
            t_wh2v = t_wh2[:].rearrange("p (b k) -> p b k", k=2)
            nc.gpsimd.tensor_scalar_mul(t_wh2v, T8v[:, :, 2:4], 0.5)
            t_pt2 = sb.tile([NN, 2], f32)    # [pa, ta]
            nc.gpsimd.tensor_tensor(
                t_pt2[:].rearrange("p (b o) -> p b o", o=1),
                T8v[:, :, 2:3], T8v[:, :, 3:4], OP.mult)
            t_s1 = sb.tile([NN, 1], f32)
            nc.gpsimd.tensor_tensor(t_s1[:], t_pt2[:, 0:1], t_pt2[:, 1:2],
                                    OP.add)
            # Q after s1, so X1's single [Pool>=Qhi] wait covers s1 too
            t_Q = sb.tile([NN, 8], f32)
            nc.gpsimd.tensor_tensor(
                t_Q[:, 0:4].rearrange("p (b k) -> p b k", k=2),
                T8v[:, :, 0:2], t_wh2v, OP.subtract)
            nc.gpsimd.tensor_tensor(
                t_Q[:, 4:8].rearrange("p (b k) -> p b k", k=2),
                T8v[:, :, 0:2], t_wh2v, OP.add)

            # DVE: X1 = [i1 | e2], X2 = [e1 | i2]  (min/max is DVE-only)
            Qh = t_Q[:].rearrange("p (h x) -> p h x", h=2)
            t_X1 = sb.tile([NN, 4], f32)
            nc.vector.tensor_tensor(
                t_X1[:].rearrange("p (h k) -> p h k", k=2),
                Qh[:, :, 0:2], Qh[:, :, 2:4], OP.max)
            t_X2 = sb.tile([NN, 4], f32)
            nc.vector.tensor_tensor(
                t_X2[:].rearrange("p (h k) -> p h k", k=2),
                Qh[:, :, 0:2], Qh[:, :, 2:4], OP.min)
            # W2 = [iwc_x iwc_y ew_x ew_y]
            t_iw = sb.tile([NN, 2], f32)
            nc.vector.tensor_tensor(t_iw[:], t_X2[:, 2:4], t_X1[:, 0:2],
                                    OP.subtract)
            t_W2 = sb.tile([NN, 4], f32)
            nc.vector.tensor_single_scalar(t_W2[:, 0:2], t_iw[:], 0.0, OP.max)
            nc.vector.tensor_tensor(t_W2[:, 2:4], t_X1[:, 2:4], t_X2[:, 0:2],
                                    OP.subtract)
            # ie = [inter, enc]
            t_ie = sb.tile([NN, 2], f32)
            W2v = t_W2[:].rearrange("p (x y) -> p x y", y=2)
            nc.vector.tensor_tensor(
                t_ie[:].rearrange("p (x o) -> p x o", o=1),
                W2v[:, :, 0:1], W2v[:, :, 1:2], OP.mult)
            t_d2 = sb.tile([NN, 2], f32)     # [union, enc]
            nc.vector.tensor_tensor(t_d2[:, 0:1], t_s1[:], t_ie[:, 0:1],
                                    OP.subtract)
            nc.vector.tensor_copy(t_d2[:, 1:2], t_ie[:, 1:2])
            t_d2a = sb.tile([NN, 2], f32)
            nc.vector.tensor_single_scalar(t_d2a[:], t_d2[:], 1e-6, OP.add)
            t_r2 = sb.tile([NN, 2], f32)
            nc.vector.reciprocal(t_r2[:], t_d2a[:])
            t_iou = sb.tile([NN, 1], f32)
            nc.vector.tensor_tensor(t_iou[:], t_ie[:, 0:1], t_r2[:, 0:1],
                                    OP.mult)
            t_em = sb.tile([NN, 1], f32)
            nc.vector.tensor_tensor(t_em[:], t_ie[:, 1:2], t_d2[:, 0:1],
                                    OP.subtract)
            t_q = sb.tile([NN, 1], f32)
            nc.vector.tensor_tensor(t_q[:], t_em[:], t_r2[:, 1:2], OP.mult)
            t_gi = sb.tile([NN, 1], f32)
            nc.vector.tensor_tensor(t_gi[:], t_iou[:], t_q[:], OP.subtract)

            # ---------------- R finalization (DVE only) & writeback -------
            # tm = clip(1 - clip(gi,-1,1), 0) == clip(1-gi, 0, 2)
            t_h1 = sb.tile([NN, 1], f32)
            nc.vector.tensor_scalar(t_h1[:], t_gi[:], -1.0, 1.0, OP.mult,
                                    OP.add)
            nc.vector.tensor_scalar(t_R[0:NN, 0:1], t_h1[:], 0.0, 2.0,
                                    OP.max, OP.min)
            nc.vector.tensor_tensor(t_R[0:NN, 2:3], t_co[:], t_va, OP.mult)
            ps_out = ps.tile([4, 4], f32)
            nc.tensor.matmul(ps_out[:], t_R[:], t_indD[:])
            t_os = sb.tile([4, 4], f32)
            nc.vector.tensor_copy(t_os[:], ps_out[:])
            nc.sync.dma_start(osum[:], t_os[:])

    nc.finalize()
    for blk in nc.m.functions[0].blocks:
        for ins in blk.instructions:
            si = ins.sync_info
            nw = len(si.on_wait) if (si and si.on_wait) else 0
            cap = 2 if type(ins).__name__ == "InstDMACopy" else 1
            if nw > cap:
                import os as _os
                if _os.environ.get("BASSDL_NO_WAIT_ASSERT"):
                    print("WAITVIOLATION", type(ins).__name__, ins.name,
                          ins.engine, [x.ant_name for x in si.on_wait])
                else:
                    raise AssertionError(
                        f"{type(ins).__name__} {ins.name} has {nw} sync waits "
                        f"(cap {cap} in this walrus build) — restructure deps")
    return nc


def host_prep(objectness, boxes, classes, gt_boxes, gt_labels):
    """Build the 8 per-core input maps.  Index/one-hot prep from gt_* plus
    pure layout transforms (transposes) of the float inputs — no float
    loss math happens here."""
    objectness = np.ascontiguousarray(np.asarray(objectness, dtype=np.float32))
    boxes = np.asarray(boxes, dtype=np.float32)
    classes = np.asarray(classes, dtype=np.float32)
    gb = np.asarray(gt_boxes, dtype=np.float32)
    gl = np.asarray(gt_labels).astype(np.int64)

    cx = np.clip((gb[:, :, 0] * np.float32(W)).astype(np.int32), 0, W - 1)
    cy = np.clip((gb[:, :, 1] * np.float32(H)).astype(np.int32), 0, H - 1)
    s = (cy * W + cx).astype(np.int64)                      # [B,N]
    eq = s[:, :, None] == s[:, None, :]                     # [B,N,N]
    tril = np.tril(np.ones((N, N), dtype=bool), k=-1)
    rank = (eq & tril[None]).sum(axis=2)                    # [B,N]
    valid = rank < M
    slot_t = np.minimum(rank, M - 1)

    # cold params
    cold = np.zeros((128, COLD), np.float32)
    for i in range(BC):
        cold[N * i:N * (i + 1), 64 + i] = 1.0               # ind20
        cold[64 * i:64 * (i + 1), 66 + i] = -1.0            # ind_neg

    in_maps = []
    for c in range(NCORES):
        bsel = slice(BC * c, BC * (c + 1))
        sB = s[bsel]                                        # [BC,N]
        il = np.arange(BC, dtype=np.int64)[:, None]
        pidx = (il * HW + sB).reshape(NN).astype(np.int32)

        glc = gl[bsel].reshape(NN)
        ohc = np.zeros((NN, C), np.float32)
        ohc[np.arange(NN), glc] = 1.0
        al = np.where(glc == 0, np.float32(ALPHA), np.float32(1 - ALPHA))
        va = valid[bsel].reshape(NN).astype(np.float32)
        oht = np.zeros((NN, M), np.float32)
        oht[np.arange(NN), slot_t[bsel].reshape(NN)] = 1.0

        hot = np.zeros((NN, HOT), np.float32)
        hot[:, 0] = pidx.view(np.float32)
        hot[:, 1:5] = gb[bsel].reshape(NN, 4)
        hot[:, 5:13] = oht
        hot[:, 13] = va
        hot[:, 14] = al
        hot[:, 15:23] = (np.arange(M) + 1000.0).astype(np.float32)[None, :]
        hot[:, 23:31] = np.arange(M, dtype=np.float32)[None, :]

        coldc = cold.copy()
        coldc[0:NN, 0:64] = ohc

        pk = np.empty((BC, HW, PK), np.float32)
        pk[:, :, 0:8] = objectness[bsel].transpose(0, 2, 3, 1).reshape(
            BC, HW, M)
        pk[:, :, 8:40] = boxes[bsel].transpose(0, 3, 4, 1, 2).reshape(
            BC, HW, M * 4)
        pk[:, :, 40:PK] = classes[bsel].transpose(0, 3, 4, 2, 1).reshape(
            BC, HW, C * M)

        in_maps.append({
            "obj": objectness[bsel].reshape(-1),
            "pack": pk.reshape(-1),
            "ph": hot,
            "pc": coldc,
        })
    return in_maps


def assemble(results):
    """Unshard: per-core [4,4] sums -> three weighted scalar means."""
    box, cls_, objl = [], [], []
    for r in results:
        o = np.asarray(r["osum"], dtype=np.float32)
        for i in range(BC):
            box.append(o[0, i] / np.float32(N))
            cls_.append(o[1, i] / np.float32(N))
            objl.append((o[2, i] + o[3, 2 + i]) / np.float32(M * HW))
    bl = np.float32(np.sum(np.asarray(box, np.float32)) / np.float32(B))
    cl = np.float32(np.sum(np.asarray(cls_, np.float32)) / np.float32(B))
    ol = np.float32(np.sum(np.asarray(objl, np.float32)) / np.float32(B))
    return (np.float32(bl * np.float32(BOX_W)),
            np.float32(cl * np.float32(CLS_W)),
            np.float32(ol * np.float32(OBJ_W)))


def _get_program():
    global _PROG
    if _PROG is None:
        _PROG = build_program()
    return _PROG


LAST_RESULTS = None  # BassKernelResults of the most recent run (for test.py)


def kernel(objectness, boxes, classes, gt_boxes, gt_labels):
    import os
    from concourse.bass_utils import run_bass_kernel_spmd

    global LAST_RESULTS
    nc = _get_program()
    in_maps = host_prep(objectness, boxes, classes, gt_boxes, gt_labels)
    trace = bool(os.environ.get("BASSDL_TRACE"))
    res = run_bass_kernel_spmd(nc, in_maps, list(range(NCORES)), trace=trace)
    LAST_RESULTS = res
    return assemble(res.results)


# revision 32
# speedup vs baseline: 1.2051x; 1.0238x over previous
"""Trainium2 Bass kernel for nn_DetectionLoss (B=16, M=8, H=W=112, C=64, N=20).

Strategy (pure data parallel over batch, 2 images per core on 8 cores):

  - The objectness BCE negative term sum(ln(1-p)) over all M*H*W cells is
    streamed through the Scalar engine (Ln activation with fused free-axis
    accumulation) — the only full-tensor pass.
  - Everything else touches only the <=20 GT cells per image.  The HW
    indirect-DMA gather semantics here are: ONE offset per destination
    partition row, reading a PHYSICALLY CONTIGUOUS run from the source
    (src-view strides are ignored; `coef` = product of src dims after the
    offset axis scales the offset).  So the host PRE-PACKS a per-cell
    tensor pack[i, s, 0:552] = [obj scores (m) | boxes (m,k) | classes
    (m,c)] and a single indirect DMA with host-known offsets (i*HW+s)
    lands each GT's full working set in one SBUF partition row.
  - Slot selection (first m with score>0.5 else 0) and the slot-dependent
    selection of boxes/logits run on device via is_equal masks against
    host-provided m-grids, so no second (device-offset) gather round-trip
    is needed.
  - GIoU runs on the Pool engine, focal CE glue on DVE, exp/ln on Scalar,
    all overlapping; one fused TensorE matmul against 0/1 indicator
    columns produces all per-image sums in a single [4,4] PSUM tile.
  - Host work is limited to integer/index/layout prep (transposes of the
    input tensors, one-hots from gt_labels, cell indices from gt_boxes)
    and the final 16->3 reduction; all floating-point loss math over the
    input values runs on device.

Sync-wait discipline (this walrus build encodes at most 1 wait on compute
instructions, 2 on DMA): per engine, the first consumer of each DMA is
ordered so every instruction adds at most one new semaphore wait.
"""
import sys

if "/opt/trn_rl_repo" not in sys.path:
    sys.path.insert(0, "/opt/trn_rl_repo")

import numpy as np

B, M, H, W, C, N = 16, 8, 112, 112, 64, 20
NCORES = 8
BC = B // NCORES          # images per core
NN = BC * N               # gt rows per core
HW = H * W                # 12544
OBJ_TOT = BC * M * HW     # 200704 = 128 * 1568
FREE = OBJ_TOT // 128     # 1568
NT = 2                    # column tiles for the objectness stream
FW = FREE // NT

PK = 8 + M * 4 + M * C    # 552 pack columns per cell
PACK_TOT = BC * HW * PK

POS_W = 10.0
ALPHA = 0.25
EPS = 1e-7
OBJ_W, BOX_W, CLS_W = 0.1, 1.0, 1.0

HOT = 31                  # hot cols: pidx|gt4|oh_t|valid|alpha|m1000|mgrid8
COLD = 68                 # cold: ohc64|ind4

_PROG = None


def _install_drain_patch():
    """This walrus build only encodes a limited number of sync waits on the
    CTRL (drain) instruction; Tile's end-of-kernel drain can exceed it.
    Split the waits across a chain of single-wait SP nops instead."""
    import concourse.tile as tile_mod
    import concourse.mybir as mb
    from concourse.vector_clock import ScopedClock

    if getattr(tile_mod.TileContext, "_drain_patch_installed", False):
        return

    def _patched(self, tick_clock, wait_clock):
        nc = self.nc
        probe = nc.engines[mb.EngineType.SP].nop()
        wait_clock.add_sem_waits(
            probe.ins, ScopedClock({None: tick_clock.global_clock})
        )
        si = probe.ins.sync_info
        waits = list(si.on_wait) if (si is not None and si.on_wait) else []
        if len(waits) > 1:
            probe.ins.sync_info = mb.SyncInfo(
                on_wait=[waits[0]], on_update=si.on_update
            )
            for w in waits[1:]:
                extra = nc.engines[mb.EngineType.SP].nop()
                extra.ins.sync_info = mb.SyncInfo(on_wait=[w], on_update=[])
        nc.sync.drain()

        nc.all_engine_barrier()
        assert self.sems is not None
        popped = nc._tile_sem_poison_stack.pop()
        assert popped is self._sem_poison
        nc.clear_and_free_semaphores(list(self.sems.allocated().values()))
        nc.all_engine_barrier()

    tile_mod.TileContext._drain_and_barrier = _patched
    tile_mod.TileContext._drain_patch_installed = True


def build_program():
    import concourse.bass as bass
    import concourse.mybir as mybir
    import concourse.tile as tile

    _install_drain_patch()
    dt = mybir.dt
    AF = mybir.ActivationFunctionType
    OP = mybir.AluOpType
    AX = mybir.AxisListType.X

    nc = bass.Bass()
    f32, i32 = dt.float32, dt.int32
    obj = nc.declare_dram_parameter("obj", [OBJ_TOT], f32, isOutput=False)
    pack = nc.declare_dram_parameter("pack", [PACK_TOT], f32, isOutput=False)
    ph = nc.declare_dram_parameter("ph", [NN, HOT], f32, isOutput=False)
    pcold = nc.declare_dram_parameter("pc", [128, COLD], f32, isOutput=False)
    osum = nc.declare_dram_parameter("osum", [4, 4], f32, isOutput=True)

    IOff = bass.IndirectOffsetOnAxis
    packv = pack.rearrange("(x c) -> x c", c=PK)       # coef = PK on axis 0
    objv = obj.rearrange("(p f) -> p f", p=128)

    with tile.TileContext(nc) as tc:
        with (
            tc.tile_pool(name="sb", bufs=1) as sb,
            tc.tile_pool(name="ps", bufs=1, space="PSUM") as ps,
        ):
            # ---------------- t0: DMAs, memsets, act-table preload --------
            # issues spread across engine sequencers so the transfers land
            # on parallel queues and hot (the gather's dep) goes first
            # hot issues first and alone on SP so its queue drains
            # immediately; the big stream DMAs are issued from Pool AFTER
            # the gather instruction so their transfers cannot crowd the
            # gather's packets out of the DMA engines
            t_ph = sb.tile([NN, HOT], f32)
            nc.sync.dma_start(t_ph[:], ph[:])
            t_pc = sb.tile([128, COLD], f32)
            nc.scalar.dma_start(t_pc[:], pcold[:])
            t_str = [sb.tile([128, FW], f32, name=f"t_str{t}")
                     for t in range(NT)]

            t_R = sb.tile([128, 4], f32)
            nc.vector.memset(t_R[:], 0.0)
            t_dmy = sb.tile([1, 1], f32)
            nc.gpsimd.memset(t_dmy[:], 0.0)
            t_dmy2 = sb.tile([1, 1], f32)
            # early dummy activation: forces the (single) Ln/Exp act-table
            # load to overlap the input DMAs instead of the critical path
            nc.scalar.activation(t_dmy2[:], t_dmy[:], AF.Exp)

            # hot param views
            t_gt = t_ph[:, 1:5]
            t_oht = t_ph[:, 5:13]
            t_va = t_ph[:, 13:14]
            t_al = t_ph[:, 14:15]
            t_m1000 = t_ph[:, 15:23]
            t_mg8 = t_ph[:, 23:31]
            # cold param views
            t_ohc = t_pc[0:NN, 0:64]
            t_ind = t_pc[:, 64:68]

            # ---------------- the one gather (Pool) -----------------------
            t_pack = sb.tile([NN, PK], f32)
            nc.gpsimd.indirect_dma_start(
                t_pack[:], None, packv,
                IOff(ap=t_ph[:, 0:1].bitcast(i32), axis=0),
            )
            t_sc = t_pack[:, 0:8]
            t_bx = t_pack[:, 8:40]
            t_cl = t_pack[:, 40:PK]
            # stream DMAs issued behind the gather on Pool (see above)
            nc.gpsimd.dma_start(t_str[0][:], objv[:, 0:FW])
            nc.gpsimd.dma_start(t_str[1][:], objv[:, FW:2 * FW])

            # ---------------- objectness stream (Scal) --------------------
            # separate accum tiles: a shared one would add a same-engine
            # WAW semaphore wait on top of the chunk-DMA wait (cap 1)
            t_acc0 = sb.tile([128, 1], f32)
            t_acc1 = sb.tile([128, 1], f32)
            t_staccs = [t_acc0, t_acc1]
            t_strouts = [sb.tile([128, FW], f32, name=f"t_strout{t}")
                         for t in range(NT)]
            for t in range(NT):
                nc.scalar.activation(
                    t_strouts[t][:], t_str[t][:], AF.Ln, scale=-1.0, bias=1.0,
                    accum_out=t_staccs[t][:],
                )

            # ---------------- slot chain (DVE) ----------------------------
            # T8 cols: [p_cx p_cy p_w p_h | t_cx t_cy t_w t_h]
            T8 = sb.tile([NN, 8], f32)
            t_sel = sb.tile([NN, M], f32)
            nc.vector.tensor_single_scalar(t_sel[:], t_sc, 0.5, OP.is_gt)
            nc.vector.tensor_copy(T8[:, 4:8], t_gt)   # observes hot DMA
            t_v = sb.tile([NN, M], f32)
            nc.vector.scalar_tensor_tensor(
                t_v[:], t_sel[:], -1000.0, t_m1000, OP.mult, OP.add)
            t_ft = sb.tile([NN, 1], f32)
            nc.vector.tensor_reduce(t_ft[:], t_v[:], AX, OP.min)
            # slot = ft * (ft < 900) in one op
            t_slot = sb.tile([NN, 1], f32)
            nc.vector.scalar_tensor_tensor(
                t_slot[:], t_ft[:], 900.0, t_ft[:], OP.is_lt, OP.mult)
            # ppos = scores . onehot(slot_t)  (head of the positive-cell
            # correction; the Pool-side product doubles as Pool's observer
            # of the gather DMA, the tiny reduce runs on DVE)
            t_ppj = sb.tile([NN, M], f32)
            nc.gpsimd.tensor_tensor(t_ppj[:], t_sc, t_oht, OP.mult)
            t_pp = sb.tile([NN, 1], f32)
            nc.vector.tensor_reduce(t_pp[:], t_ppj[:], AX, OP.add)

            # -------- positive-cell BCE correction tail (Pool+Scal) -------
            # corr = -10*ln(max(p,eps)) + ln(max(1-p,eps))
            t_L2 = sb.tile([NN, 2], f32)
            nc.gpsimd.tensor_single_scalar(
                t_L2[:, 0:1], t_pp[:], 1e-38, OP.max)
            t_1p = sb.tile([NN, 1], f32)
            nc.gpsimd.tensor_scalar(
                t_1p[:], t_pp[:], -1.0, 1.0, OP.mult, OP.add)
            nc.gpsimd.tensor_single_scalar(
                t_L2[:, 1:2], t_1p[:], 1e-38, OP.max)
            t_L2l = sb.tile([NN, 2], f32)
            nc.scalar.activation(t_L2l[:], t_L2[:], AF.Ln)
            t_L2c = sb.tile([NN, 2], f32)
            nc.gpsimd.tensor_single_scalar(t_L2c[:], t_L2l[:], -100.0, OP.max)
            t_l10 = sb.tile([NN, 1], f32)
            nc.gpsimd.tensor_scalar_mul(t_l10[:], t_L2c[:, 0:1], -POS_W)
            t_co = sb.tile([NN, 1], f32)
            nc.gpsimd.tensor_tensor(t_co[:], t_l10[:], t_L2c[:, 1:2], OP.add)

            # ---------------- slot one-hot + box select (DVE) -------------
            # emitted before the class path so the longer GIoU chain gets
            # scheduling priority
            t_oh8 = sb.tile([NN, M], f32)
            bm0, bm1 = bass.broadcast_tensor_aps(t_mg8, t_slot[:])
            nc.vector.tensor_tensor(t_oh8[:], bm0, bm1, OP.is_equal)
            a8 = t_oh8[:]
            oh8_c = bass.AP(a8.tensor, a8.offset,
                            [list(a8.ap[0]), [0, C], list(a8.ap[1])])
            oh8_k = bass.AP(a8.tensor, a8.offset,
                            [list(a8.ap[0]), list(a8.ap[1]), [0, 4]])
            t_m32 = sb.tile([NN, 32], f32)
            nc.vector.tensor_tensor(
                t_m32[:].rearrange("p (m k) -> p m k", k=4),
                t_bx.rearrange("p (m k) -> p m k", k=4), oh8_k, OP.mult)

            # ---------------- GIoU (Pool, bx4/recip on DVE) ---------------
            nc.vector.tensor_reduce(
                T8[:, 0:4], t_m32[:].rearrange("p (m k) -> p k m", k=4),
                AX, OP.add)

            # Pool assembles Q = [lo_p lo_t | hi_p hi_t] and the pa/ta
            # products; DVE does the min/max pairs and the divide chain
            # (overlapping the Scalar focal chain).
            T8v = T8[:].rearrange("p (b k) -> p b k", k=4)
            t_wh2 = sb.tile([NN, 4], f32)
            t_wh2v = t_wh2[:].rearrange("p (b k) -> p b k", k=2)
            nc.gpsimd.tensor_scalar_mul(t_wh2v, T8v[:, :, 2:4], 0.5)
            t_pt2 = sb.tile([NN, 2], f32)    # [pa, ta]
            nc.gpsimd.tensor_tensor(
                t_pt2[:].rearrange("p (b o) -> p b o", o=1),
                T8v[:, :, 2:3], T8v[:, :, 3:4], OP.mult)
            t_s1 = sb.tile([NN, 1], f32)
            nc.gpsimd.tensor_tensor(t_s1[:], t_pt2[:, 0:1], t_pt2[:, 1:2],
                                    OP.add)
            # Q after s1, so X1's single [Pool>=Qhi] wait covers s1 too
            t_Q = sb.tile([NN, 8], f32)
            nc.gpsimd.tensor_tensor(
                t_Q[:, 0:4].rearrange("p (b k) -> p b k", k=2),
                T8v[:, :, 0:2], t_wh2v, OP.subtract)
            nc.gpsimd.tensor_tensor(
                t_Q[:, 4:8].rearrange("p (b k) -> p b k", k=2),
                T8v[:, :, 0:2], t_wh2v, OP.add)

            # DVE: X1 = [i1 | e2], X2 = [e1 | i2]  (min/max is DVE-only)
            Qh = t_Q[:].rearrange("p (h x) -> p h x", h=2)
            t_X1 = sb.tile([NN, 4], f32)
            nc.vector.tensor_tensor(
                t_X1[:].rearrange("p (h k) -> p h k", k=2),
                Qh[:, :, 0:2], Qh[:, :, 2:4], OP.max)
            t_X2 = sb.tile([NN, 4], f32)
            nc.vector.tensor_tensor(
                t_X2[:].rearrange("p (h k) -> p h k", k=2),
                Qh[:, :, 0:2], Qh[:, :, 2:4], OP.min)
            # W2 = [iwc_x iwc_y ew_x ew_y]
            t_iw = sb.tile([NN, 2], f32)
            nc.vector.tensor_tensor(t_iw[:], t_X2[:, 2:4], t_X1[:, 0:2],
                                    OP.subtract)
            t_W2 = sb.tile([NN, 4], f32)
            nc.vector.tensor_single_scalar(t_W2[:, 0:2], t_iw[:], 0.0, OP.max)
            nc.vector.tensor_tensor(t_W2[:, 2:4], t_X1[:, 2:4], t_X2[:, 0:2],
                                    OP.subtract)
            # ie = [inter, enc]
            t_ie = sb.tile([NN, 2], f32)
            W2v = t_W2[:].rearrange("p (x y) -> p x y", y=2)
            nc.vector.tensor_tensor(
                t_ie[:].rearrange("p (x o) -> p x o", o=1),
                W2v[:, :, 0:1], W2v[:, :, 1:2], OP.mult)
            t_d2 = sb.tile([NN, 2], f32)     # [union, enc]
            nc.vector.tensor_tensor(t_d2[:, 0:1], t_s1[:], t_ie[:, 0:1],
                                    OP.subtract)
            nc.vector.tensor_copy(t_d2[:, 1:2], t_ie[:, 1:2])
            t_d2a = sb.tile([NN, 2], f32)
            nc.vector.tensor_single_scalar(t_d2a[:], t_d2[:], 1e-6, OP.add)
            # Pool assembles ne = [inter, em] while DVE runs the recip
            t_ne = sb.tile([NN, 2], f32)
            nc.gpsimd.tensor_copy(t_ne[:, 0:1], t_ie[:, 0:1])
            nc.gpsimd.tensor_tensor(t_ne[:, 1:2], t_ie[:, 1:2], t_d2[:, 0:1],
                                    OP.subtract)
            t_neD = sb.tile([NN, 2], f32)
            nc.vector.tensor_copy(t_neD[:], t_ne[:])   # Pool observer
            t_r2 = sb.tile([NN, 2], f32)
            nc.vector.reciprocal(t_r2[:], t_d2a[:])
            t_pr2 = sb.tile([NN, 2], f32)    # [iou, q]
            nc.vector.tensor_tensor(t_pr2[:], t_neD[:], t_r2[:], OP.mult)
            t_gi = sb.tile([NN, 1], f32)
            nc.vector.tensor_tensor(t_gi[:], t_pr2[:, 0:1], t_pr2[:, 1:2],
                                    OP.subtract)

            # ---------------- class logits at slot + focal (DVE/Scal) -----
            # indD doubles as the DVE cold-DMA observer (before xjunk)
            t_indD = sb.tile([128, 4], f32)
            nc.vector.tensor_copy(t_indD[:], t_ind)
            t_m512 = sb.tile([NN, M * C], f32)
            nc.vector.tensor_tensor(
                t_m512[:].rearrange("p (c m) -> p c m", m=M),
                t_cl.rearrange("p (c m) -> p c m", m=M), oh8_c, OP.mult)
            # pack classes are (c, m) so the m-reduction is contiguous
            t_log64 = sb.tile([NN, C], f32)
            nc.vector.tensor_reduce(
                t_log64[:], t_m512[:].rearrange("p (c m) -> p c m", m=M),
                AX, OP.add)
            # focal CE — pt/om/sq chained on Scalar, xl parallel on DVE
            t_exp = sb.tile([NN, C], f32)
            t_se = sb.tile([NN, 1], f32)
            nc.scalar.activation(t_exp[:], t_log64[:], AF.Exp,
                                 accum_out=t_se[:])
            t_lse = sb.tile([NN, 1], f32)
            nc.scalar.activation(t_lse[:], t_se[:], AF.Ln)
            t_xjunk = sb.tile([NN, C], f32)
            nc.vector.tensor_tensor(t_xjunk[:], t_log64[:], t_ohc, OP.mult)
            t_xl = sb.tile([NN, 1], f32)
            nc.vector.tensor_reduce(t_xl[:], t_xjunk[:], AX, OP.add)
            t_lsec = sb.tile([NN, 1], f32)
            nc.vector.tensor_copy(t_lsec[:], t_lse[:])   # Act observer
            t_ce = sb.tile([NN, 1], f32)
            nc.vector.tensor_tensor(t_ce[:], t_lsec[:], t_xl[:], OP.subtract)
            t_pt = sb.tile([NN, 1], f32)
            nc.scalar.activation(t_pt[:], t_ce[:], AF.Exp, scale=-1.0)
            t_om = sb.tile([NN, 1], f32)
            nc.scalar.activation(t_om[:], t_pt[:], AF.Copy, scale=-1.0,
                                 bias=1.0 - EPS)
            t_sq = sb.tile([NN, 1], f32)
            nc.scalar.activation(t_sq[:], t_om[:], AF.Square)
            # f1 = ce * (1-pt)^2 via AP-scale on Scalar (ce tick already
            # observed by Scal at pt) so Rcls adds a single new wait
            t_f1 = sb.tile([NN, 1], f32)
            nc.scalar.activation(t_f1[:], t_ce[:], AF.Identity,
                                 scale=t_sq[:])
            nc.vector.tensor_tensor(t_R[0:NN, 1:2], t_f1[:], t_al, OP.mult)
            # stream sums -> R col 3
            nc.vector.tensor_tensor(
                t_R[:, 3:4], t_acc0[:], t_acc1[:], OP.add)

            # ---------------- R finalization (DVE only) & writeback -------
            # tm = clip(1 - clip(gi,-1,1), 0) == clip(1-gi, 0, 2)
            t_h1 = sb.tile([NN, 1], f32)
            nc.vector.tensor_scalar(t_h1[:], t_gi[:], -1.0, 1.0, OP.mult,
                                    OP.add)
            nc.vector.tensor_scalar(t_R[0:NN, 0:1], t_h1[:], 0.0, 2.0,
                                    OP.max, OP.min)
            nc.vector.tensor_tensor(t_R[0:NN, 2:3], t_co[:], t_va, OP.mult)
            ps_out = ps.tile([4, 4], f32)
            nc.tensor.matmul(ps_out[:], t_R[:], t_indD[:])
            t_os = sb.tile([4, 4], f32)
            nc.vector.tensor_copy(t_os[:], ps_out[:])
            nc.sync.dma_start(osum[:], t_os[:])

    nc.finalize()
    for blk in nc.m.functions[0].blocks:
        for ins in blk.instructions:
            si = ins.sync_info
            nw = len(si.on_wait) if (si and si.on_wait) else 0
            cap = 2 if type(ins).__name__ == "InstDMACopy" else 1
            if nw > cap:
                import os as _os
                if _os.environ.get("BASSDL_NO_WAIT_ASSERT"):
                    print("WAITVIOLATION", type(ins).__name__, ins.name,
                          ins.engine, [x.ant_name for x in si.on_wait])
                else:
                    raise AssertionError(
                        f"{type(ins).__name__} {ins.name} has {nw} sync waits "
                        f"(cap {cap} in this walrus build) — restructure deps")
    return nc


def host_prep(objectness, boxes, classes, gt_boxes, gt_labels):
    """Build the 8 per-core input maps.  Index/one-hot prep from gt_* plus
    pure layout transforms (transposes) of the float inputs — no float
    loss math happens here."""
    objectness = np.ascontiguousarray(np.asarray(objectness, dtype=np.float32))
    boxes = np.asarray(boxes, dtype=np.float32)
    classes = np.asarray(classes, dtype=np.float32)
    gb = np.asarray(gt_boxes, dtype=np.float32)
    gl = np.asarray(gt_labels).astype(np.int64)

    cx = np.clip((gb[:, :, 0] * np.float32(W)).astype(np.int32), 0, W - 1)
    cy = np.clip((gb[:, :, 1] * np.float32(H)).astype(np.int32), 0, H - 1)
    s = (cy * W + cx).astype(np.int64)                      # [B,N]
    eq = s[:, :, None] == s[:, None, :]                     # [B,N,N]
    tril = np.tril(np.ones((N, N), dtype=bool), k=-1)
    rank = (eq & tril[None]).sum(axis=2)                    # [B,N]
    valid = rank < M
    slot_t = np.minimum(rank, M - 1)

    # cold params
    cold = np.zeros((128, COLD), np.float32)
    for i in range(BC):
        cold[N * i:N * (i + 1), 64 + i] = 1.0               # ind20
        cold[64 * i:64 * (i + 1), 66 + i] = -1.0            # ind_neg

    in_maps = []
    for c in range(NCORES):
        bsel = slice(BC * c, BC * (c + 1))
        sB = s[bsel]                                        # [BC,N]
        il = np.arange(BC, dtype=np.int64)[:, None]
        pidx = (il * HW + sB).reshape(NN).astype(np.int32)

        glc = gl[bsel].reshape(NN)
        ohc = np.zeros((NN, C), np.float32)
        ohc[np.arange(NN), glc] = 1.0
        al = np.where(glc == 0, np.float32(ALPHA), np.float32(1 - ALPHA))
        va = valid[bsel].reshape(NN).astype(np.float32)
        oht = np.zeros((NN, M), np.float32)
        oht[np.arange(NN), slot_t[bsel].reshape(NN)] = 1.0

        hot = np.zeros((NN, HOT), np.float32)
        hot[:, 0] = pidx.view(np.float32)
        hot[:, 1:5] = gb[bsel].reshape(NN, 4)
        hot[:, 5:13] = oht
        hot[:, 13] = va
        hot[:, 14] = al
        hot[:, 15:23] = (np.arange(M) + 1000.0).astype(np.float32)[None, :]
        hot[:, 23:31] = np.arange(M, dtype=np.float32)[None, :]

        coldc = cold.copy()
        coldc[0:NN, 0:64] = ohc

        pk = np.empty((BC, HW, PK), np.float32)
        pk[:, :, 0:8] = objectness[bsel].transpose(0, 2, 3, 1).reshape(
            BC, HW, M)
        pk[:, :, 8:40] = boxes[bsel].transpose(0, 3, 4, 1, 2).reshape(
            BC, HW, M * 4)
        pk[:, :, 40:PK] = classes[bsel].transpose(0, 3, 4, 2, 1).reshape(
            BC, HW, C * M)

        in_maps.append({
            "obj": objectness[bsel].reshape(-1),
            "pack": pk.reshape(-1),
            "ph": hot,
            "pc": coldc,
        })
    return in_maps


def assemble(results):
    """Unshard: per-core [4,4] sums -> three weighted scalar means."""
    box, cls_, objl = [], [], []
    for r in results:
        o = np.asarray(r["osum"], dtype=np.float32)
        for i in range(BC):
            box.append(o[0, i] / np.float32(N))
            cls_.append(o[1, i] / np.float32(N))
            objl.append((o[2, i] + o[3, 2 + i]) / np.float32(M * HW))
    bl = np.float32(np.sum(np.asarray(box, np.float32)) / np.float32(B))
    cl = np.float32(np.sum(np.asarray(cls_, np.float32)) / np.float32(B))
    ol = np.float32(np.sum(np.asarray(objl, np.float32)) / np.float32(B))
    return (np.float32(bl * np.float32(BOX_W)),
            np.float32(cl * np.float32(CLS_W)),
            np.float32(ol * np.float32(OBJ_W)))


def _get_program():
    global _PROG
    if _PROG is None:
        _PROG = build_program()
    return _PROG


LAST_RESULTS = None  # BassKernelResults of the most recent run (for test.py)


def kernel(objectness, boxes, classes, gt_boxes, gt_labels):
    import os
    from concourse.bass_utils import run_bass_kernel_spmd

    global LAST_RESULTS
    nc = _get_program()
    in_maps = host_prep(objectness, boxes, classes, gt_boxes, gt_labels)
    trace = bool(os.environ.get("BASSDL_TRACE"))
    res = run_bass_kernel_spmd(nc, in_maps, list(range(NCORES)), trace=trace)
    LAST_RESULTS = res
    return assemble(res.results)


# revision 33
# speedup vs baseline: 1.2238x; 1.0155x over previous
"""Trainium2 Bass kernel for nn_DetectionLoss (B=16, M=8, H=W=112, C=64, N=20).

Strategy (pure data parallel over batch, 2 images per core on 8 cores):

  - The objectness BCE negative term sum(ln(1-p)) over all M*H*W cells is
    streamed through the Scalar engine (Ln activation with fused free-axis
    accumulation) — the only full-tensor pass.
  - Everything else touches only the <=20 GT cells per image.  The HW
    indirect-DMA gather semantics here are: ONE offset per destination
    partition row, reading a PHYSICALLY CONTIGUOUS run from the source
    (src-view strides are ignored; `coef` = product of src dims after the
    offset axis scales the offset).  So the host PRE-PACKS a per-cell
    tensor pack[i, s, 0:552] = [obj scores (m) | boxes (m,k) | classes
    (m,c)] and a single indirect DMA with host-known offsets (i*HW+s)
    lands each GT's full working set in one SBUF partition row.
  - Slot selection (first m with score>0.5 else 0) and the slot-dependent
    selection of boxes/logits run on device via is_equal masks against
    host-provided m-grids, so no second (device-offset) gather round-trip
    is needed.
  - GIoU runs on the Pool engine, focal CE glue on DVE, exp/ln on Scalar,
    all overlapping; one fused TensorE matmul against 0/1 indicator
    columns produces all per-image sums in a single [4,4] PSUM tile.
  - Host work is limited to integer/index/layout prep (transposes of the
    input tensors, one-hots from gt_labels, cell indices from gt_boxes)
    and the final 16->3 reduction; all floating-point loss math over the
    input values runs on device.

Sync-wait discipline (this walrus build encodes at most 1 wait on compute
instructions, 2 on DMA): per engine, the first consumer of each DMA is
ordered so every instruction adds at most one new semaphore wait.
"""
import sys

if "/opt/trn_rl_repo" not in sys.path:
    sys.path.insert(0, "/opt/trn_rl_repo")

import numpy as np

B, M, H, W, C, N = 16, 8, 112, 112, 64, 20
NCORES = 8
BC = B // NCORES          # images per core
NN = BC * N               # gt rows per core
HW = H * W                # 12544
OBJ_TOT = BC * M * HW     # 200704 = 128 * 1568
FREE = OBJ_TOT // 128     # 1568
NT = 2                    # column tiles for the objectness stream
FW = FREE // NT

PK = 8 + M * 4 + M * C    # 552 pack columns per cell
PACK_TOT = BC * HW * PK

POS_W = 10.0
ALPHA = 0.25
EPS = 1e-7
OBJ_W, BOX_W, CLS_W = 0.1, 1.0, 1.0

HOT = 31                  # hot cols: pidx|gt4|oh_t|valid|alpha|m1000|mgrid8
COLD = 68                 # cold: ohc64|ind4

_PROG = None


def _install_drain_patch():
    """This walrus build only encodes a limited number of sync waits on the
    CTRL (drain) instruction; Tile's end-of-kernel drain can exceed it.
    Split the waits across a chain of single-wait SP nops instead."""
    import concourse.tile as tile_mod
    import concourse.mybir as mb
    from concourse.vector_clock import ScopedClock

    if getattr(tile_mod.TileContext, "_drain_patch_installed", False):
        return

    def _patched(self, tick_clock, wait_clock):
        nc = self.nc
        probe = nc.engines[mb.EngineType.SP].nop()
        wait_clock.add_sem_waits(
            probe.ins, ScopedClock({None: tick_clock.global_clock})
        )
        si = probe.ins.sync_info
        waits = list(si.on_wait) if (si is not None and si.on_wait) else []
        if len(waits) > 1:
            probe.ins.sync_info = mb.SyncInfo(
                on_wait=[waits[0]], on_update=si.on_update
            )
            for w in waits[1:]:
                extra = nc.engines[mb.EngineType.SP].nop()
                extra.ins.sync_info = mb.SyncInfo(on_wait=[w], on_update=[])
        nc.sync.drain()

        nc.all_engine_barrier()
        assert self.sems is not None
        popped = nc._tile_sem_poison_stack.pop()
        assert popped is self._sem_poison
        nc.clear_and_free_semaphores(list(self.sems.allocated().values()))
        nc.all_engine_barrier()

    tile_mod.TileContext._drain_and_barrier = _patched
    tile_mod.TileContext._drain_patch_installed = True


def build_program():
    import concourse.bass as bass
    import concourse.mybir as mybir
    import concourse.tile as tile

    _install_drain_patch()
    dt = mybir.dt
    AF = mybir.ActivationFunctionType
    OP = mybir.AluOpType
    AX = mybir.AxisListType.X

    nc = bass.Bass()
    f32, i32 = dt.float32, dt.int32
    obj = nc.declare_dram_parameter("obj", [OBJ_TOT], f32, isOutput=False)
    pack = nc.declare_dram_parameter("pack", [PACK_TOT], f32, isOutput=False)
    ph = nc.declare_dram_parameter("ph", [NN, HOT], f32, isOutput=False)
    pcold = nc.declare_dram_parameter("pc", [128, COLD], f32, isOutput=False)
    osum = nc.declare_dram_parameter("osum", [4, 4], f32, isOutput=True)

    IOff = bass.IndirectOffsetOnAxis
    packv = pack.rearrange("(x c) -> x c", c=PK)       # coef = PK on axis 0
    objv = obj.rearrange("(p f) -> p f", p=128)

    with tile.TileContext(nc) as tc:
        with (
            tc.tile_pool(name="sb", bufs=1) as sb,
            tc.tile_pool(name="ps", bufs=1, space="PSUM") as ps,
        ):
            # ---------------- t0: DMAs, memsets, act-table preload --------
            # issues spread across engine sequencers so the transfers land
            # on parallel queues and hot (the gather's dep) goes first
            # hot issues first and alone on SP so its queue drains
            # immediately; the big stream DMAs are issued from Pool AFTER
            # the gather instruction so their transfers cannot crowd the
            # gather's packets out of the DMA engines
            t_ph = sb.tile([NN, HOT], f32)
            nc.sync.dma_start(t_ph[:], ph[:])
            t_pc = sb.tile([128, COLD], f32)
            nc.scalar.dma_start(t_pc[:], pcold[:])
            # one shared tile for [str0 | pack | str1]: the WAW chain pins
            # the Pool-queue order str0 -> gather -> str1, putting the
            # critical gather packets ahead of half the stream traffic
            t_big = sb.tile([128, FW + PK + FW], f32)

            t_R = sb.tile([128, 4], f32)
            nc.vector.memset(t_R[:], 0.0)
            t_dmy = sb.tile([1, 1], f32)
            nc.gpsimd.memset(t_dmy[:], 0.0)
            t_dmy2 = sb.tile([1, 1], f32)
            # early dummy activation: forces the (single) Ln/Exp act-table
            # load to overlap the input DMAs instead of the critical path
            nc.scalar.activation(t_dmy2[:], t_dmy[:], AF.Exp)

            # hot param views
            t_gt = t_ph[:, 1:5]
            t_oht = t_ph[:, 5:13]
            t_va = t_ph[:, 13:14]
            t_al = t_ph[:, 14:15]
            t_m1000 = t_ph[:, 15:23]
            t_mg8 = t_ph[:, 23:31]
            # cold param views
            t_ohc = t_pc[0:NN, 0:64]
            t_ind = t_pc[:, 64:68]

            # ---------------- the one gather (Pool) -----------------------
            nc.gpsimd.dma_start(t_big[:, 0:FW], objv[:, 0:FW])
            t_pack = t_big[0:NN, FW:FW + PK]
            nc.gpsimd.indirect_dma_start(
                t_pack, None, packv,
                IOff(ap=t_ph[:, 0:1].bitcast(i32), axis=0),
            )
            nc.gpsimd.dma_start(t_big[:, FW + PK:FW + PK + FW],
                                objv[:, FW:2 * FW])
            t_sc = t_big[0:NN, FW:FW + 8]
            t_bx = t_big[0:NN, FW + 8:FW + 40]
            t_cl = t_big[0:NN, FW + 40:FW + PK]

            # ---------------- objectness stream (Scal) --------------------
            # separate accum tiles: a shared one would add a same-engine
            # WAW semaphore wait on top of the chunk-DMA wait (cap 1)
            t_acc0 = sb.tile([128, 1], f32)
            t_acc1 = sb.tile([128, 1], f32)
            t_staccs = [t_acc0, t_acc1]
            t_strouts = [sb.tile([128, FW], f32, name=f"t_strout{t}")
                         for t in range(NT)]
            str_srcs = [t_big[:, 0:FW], t_big[:, FW + PK:FW + PK + FW]]
            for t in range(NT):
                nc.scalar.activation(
                    t_strouts[t][:], str_srcs[t], AF.Ln, scale=-1.0, bias=1.0,
                    accum_out=t_staccs[t][:],
                )

            # ---------------- slot chain (DVE) ----------------------------
            # T8 cols: [p_cx p_cy p_w p_h | t_cx t_cy t_w t_h]
            T8 = sb.tile([NN, 8], f32)
            t_sel = sb.tile([NN, M], f32)
            nc.vector.tensor_single_scalar(t_sel[:], t_sc, 0.5, OP.is_gt)
            nc.vector.tensor_copy(T8[:, 4:8], t_gt)   # observes hot DMA
            t_v = sb.tile([NN, M], f32)
            nc.vector.scalar_tensor_tensor(
                t_v[:], t_sel[:], -1000.0, t_m1000, OP.mult, OP.add)
            t_ft = sb.tile([NN, 1], f32)
            nc.vector.tensor_reduce(t_ft[:], t_v[:], AX, OP.min)
            # slot = ft * (ft < 900) in one op
            t_slot = sb.tile([NN, 1], f32)
            nc.vector.scalar_tensor_tensor(
                t_slot[:], t_ft[:], 900.0, t_ft[:], OP.is_lt, OP.mult)
            # ppos = scores . onehot(slot_t)  (head of the positive-cell
            # correction; the Pool-side product doubles as Pool's observer
            # of the gather DMA, the tiny reduce runs on DVE)
            t_ppj = sb.tile([NN, M], f32)
            nc.gpsimd.tensor_tensor(t_ppj[:], t_sc, t_oht, OP.mult)
            t_pp = sb.tile([NN, 1], f32)
            nc.vector.tensor_reduce(t_pp[:], t_ppj[:], AX, OP.add)

            # -------- positive-cell BCE correction tail (Pool+Scal) -------
            # corr = -10*ln(max(p,eps)) + ln(max(1-p,eps))
            t_L2 = sb.tile([NN, 2], f32)
            nc.gpsimd.tensor_single_scalar(
                t_L2[:, 0:1], t_pp[:], 1e-38, OP.max)
            t_1p = sb.tile([NN, 1], f32)
            nc.gpsimd.tensor_scalar(
                t_1p[:], t_pp[:], -1.0, 1.0, OP.mult, OP.add)
            nc.gpsimd.tensor_single_scalar(
                t_L2[:, 1:2], t_1p[:], 1e-38, OP.max)
            t_L2l = sb.tile([NN, 2], f32)
            nc.scalar.activation(t_L2l[:], t_L2[:], AF.Ln)
            t_L2c = sb.tile([NN, 2], f32)
            nc.gpsimd.tensor_single_scalar(t_L2c[:], t_L2l[:], -100.0, OP.max)
            t_l10 = sb.tile([NN, 1], f32)
            nc.gpsimd.tensor_scalar_mul(t_l10[:], t_L2c[:, 0:1], -POS_W)
            t_co = sb.tile([NN, 1], f32)
            nc.gpsimd.tensor_tensor(t_co[:], t_l10[:], t_L2c[:, 1:2], OP.add)
            t_acs = sb.tile([128, 1], f32)
            nc.gpsimd.tensor_tensor(t_acs[:], t_acc0[:], t_acc1[:], OP.add)

            # ---------------- slot one-hot + box select (DVE) -------------
            # emitted before the class path so the longer GIoU chain gets
            # scheduling priority
            t_oh8 = sb.tile([NN, M], f32)
            bm0, bm1 = bass.broadcast_tensor_aps(t_mg8, t_slot[:])
            nc.vector.tensor_tensor(t_oh8[:], bm0, bm1, OP.is_equal)
            a8 = t_oh8[:]
            oh8_c = bass.AP(a8.tensor, a8.offset,
                            [list(a8.ap[0]), [0, C], list(a8.ap[1])])
            oh8_k = bass.AP(a8.tensor, a8.offset,
                            [list(a8.ap[0]), list(a8.ap[1]), [0, 4]])
            t_m32 = sb.tile([NN, 32], f32)
            nc.vector.tensor_tensor(
                t_m32[:].rearrange("p (m k) -> p m k", k=4),
                t_bx.rearrange("p (m k) -> p m k", k=4), oh8_k, OP.mult)

            # ---------------- GIoU (Pool, bx4/recip on DVE) ---------------
            nc.vector.tensor_reduce(
                T8[:, 0:4], t_m32[:].rearrange("p (m k) -> p k m", k=4),
                AX, OP.add)

            # Pool assembles Q = [lo_p lo_t | hi_p hi_t] and the pa/ta
            # products; DVE does the min/max pairs and the divide chain
            # (overlapping the Scalar focal chain).
            T8v = T8[:].rearrange("p (b k) -> p b k", k=4)
            t_wh2 = sb.tile([NN, 4], f32)
            t_wh2v = t_wh2[:].rearrange("p (b k) -> p b k", k=2)
            nc.gpsimd.tensor_scalar_mul(t_wh2v, T8v[:, :, 2:4], 0.5)
            t_pt2 = sb.tile([NN, 2], f32)    # [pa, ta]
            nc.gpsimd.tensor_tensor(
                t_pt2[:].rearrange("p (b o) -> p b o", o=1),
                T8v[:, :, 2:3], T8v[:, :, 3:4], OP.mult)
            t_s1 = sb.tile([NN, 1], f32)
            nc.gpsimd.tensor_tensor(t_s1[:], t_pt2[:, 0:1], t_pt2[:, 1:2],
                                    OP.add)
            # Q after s1, so X1's single [Pool>=Qhi] wait covers s1 too
            t_Q = sb.tile([NN, 8], f32)
            nc.gpsimd.tensor_tensor(
                t_Q[:, 0:4].rearrange("p (b k) -> p b k", k=2),
                T8v[:, :, 0:2], t_wh2v, OP.subtract)
            nc.gpsimd.tensor_tensor(
                t_Q[:, 4:8].rearrange("p (b k) -> p b k", k=2),
                T8v[:, :, 0:2], t_wh2v, OP.add)

            # DVE: X1 = [i1 | e2], X2 = [e1 | i2]  (min/max is DVE-only)
            Qh = t_Q[:].rearrange("p (h x) -> p h x", h=2)
            t_X1 = sb.tile([NN, 4], f32)
            nc.vector.tensor_tensor(
                t_X1[:].rearrange("p (h k) -> p h k", k=2),
                Qh[:, :, 0:2], Qh[:, :, 2:4], OP.max)
            t_X2 = sb.tile([NN, 4], f32)
            nc.vector.tensor_tensor(
                t_X2[:].rearrange("p (h k) -> p h k", k=2),
                Qh[:, :, 0:2], Qh[:, :, 2:4], OP.min)
            # W2 = [iwc_x iwc_y ew_x ew_y]
            t_iw = sb.tile([NN, 2], f32)
            nc.vector.tensor_tensor(t_iw[:], t_X2[:, 2:4], t_X1[:, 0:2],
                                    OP.subtract)
            t_W2 = sb.tile([NN, 4], f32)
            nc.vector.tensor_single_scalar(t_W2[:, 0:2], t_iw[:], 0.0, OP.max)
            nc.vector.tensor_tensor(t_W2[:, 2:4], t_X1[:, 2:4], t_X2[:, 0:2],
                                    OP.subtract)
            # ie = [inter, enc]
            t_ie = sb.tile([NN, 2], f32)
            W2v = t_W2[:].rearrange("p (x y) -> p x y", y=2)
            nc.vector.tensor_tensor(
                t_ie[:].rearrange("p (x o) -> p x o", o=1),
                W2v[:, :, 0:1], W2v[:, :, 1:2], OP.mult)
            t_d2 = sb.tile([NN, 2], f32)     # [union, enc]
            nc.vector.tensor_tensor(t_d2[:, 0:1], t_s1[:], t_ie[:, 0:1],
                                    OP.subtract)
            nc.vector.tensor_copy(t_d2[:, 1:2], t_ie[:, 1:2])
            t_d2a = sb.tile([NN, 2], f32)
            nc.vector.tensor_single_scalar(t_d2a[:], t_d2[:], 1e-6, OP.add)
            # Pool assembles ne = [inter, em] while DVE runs the recip
            t_ne = sb.tile([NN, 2], f32)
            nc.gpsimd.tensor_copy(t_ne[:, 0:1], t_ie[:, 0:1])
            nc.gpsimd.tensor_tensor(t_ne[:, 1:2], t_ie[:, 1:2], t_d2[:, 0:1],
                                    OP.subtract)
            t_neD = sb.tile([NN, 2], f32)
            nc.vector.tensor_copy(t_neD[:], t_ne[:])   # Pool observer
            t_r2 = sb.tile([NN, 2], f32)
            nc.vector.reciprocal(t_r2[:], t_d2a[:])
            t_pr2 = sb.tile([NN, 2], f32)    # [iou, q]
            nc.vector.tensor_tensor(t_pr2[:], t_neD[:], t_r2[:], OP.mult)
            t_gi = sb.tile([NN, 1], f32)
            nc.vector.tensor_tensor(t_gi[:], t_pr2[:, 0:1], t_pr2[:, 1:2],
                                    OP.subtract)

            # ---------------- class logits at slot + focal (DVE/Scal) -----
            # indD doubles as the DVE cold-DMA observer (before xjunk)
            t_indD = sb.tile([128, 4], f32)
            nc.vector.tensor_copy(t_indD[:], t_ind)
            t_m512 = sb.tile([NN, M * C], f32)
            nc.vector.tensor_tensor(
                t_m512[:].rearrange("p (c m) -> p c m", m=M),
                t_cl.rearrange("p (c m) -> p c m", m=M), oh8_c, OP.mult)
            # pack classes are (c, m) so the m-reduction is contiguous
            t_log64 = sb.tile([NN, C], f32)
            nc.vector.tensor_reduce(
                t_log64[:], t_m512[:].rearrange("p (c m) -> p c m", m=M),
                AX, OP.add)
            # focal CE — pt/om/sq chained on Scalar, xl parallel on DVE
            t_exp = sb.tile([NN, C], f32)
            t_se = sb.tile([NN, 1], f32)
            nc.scalar.activation(t_exp[:], t_log64[:], AF.Exp,
                                 accum_out=t_se[:])
            t_lse = sb.tile([NN, 1], f32)
            nc.scalar.activation(t_lse[:], t_se[:], AF.Ln)
            t_xjunk = sb.tile([NN, C], f32)
            nc.vector.tensor_tensor(t_xjunk[:], t_log64[:], t_ohc, OP.mult)
            t_xl = sb.tile([NN, 1], f32)
            nc.vector.tensor_reduce(t_xl[:], t_xjunk[:], AX, OP.add)
            t_lsec = sb.tile([NN, 1], f32)
            nc.vector.tensor_copy(t_lsec[:], t_lse[:])   # Act observer
            t_ce = sb.tile([NN, 1], f32)
            nc.vector.tensor_tensor(t_ce[:], t_lsec[:], t_xl[:], OP.subtract)
            t_pt = sb.tile([NN, 1], f32)
            nc.scalar.activation(t_pt[:], t_ce[:], AF.Exp, scale=-1.0)
            t_om = sb.tile([NN, 1], f32)
            nc.vector.tensor_scalar(t_om[:], t_pt[:], -1.0, 1.0 - EPS,
                                    OP.mult, OP.add)
            t_sq = sb.tile([NN, 1], f32)
            nc.vector.tensor_tensor(t_sq[:], t_om[:], t_om[:], OP.mult)
            t_f1 = sb.tile([NN, 1], f32)
            nc.vector.tensor_tensor(t_f1[:], t_sq[:], t_ce[:], OP.mult)
            nc.vector.tensor_tensor(t_R[0:NN, 1:2], t_f1[:], t_al, OP.mult)
            # stream sums summed on Pool (act2 tick observed there already),
            # funneled into R col 3 by a DVE copy
            nc.vector.tensor_copy(t_R[:, 3:4], t_acs[:])

            # ---------------- R finalization (DVE only) & writeback -------
            # tm = clip(1 - clip(gi,-1,1), 0) == clip(1-gi, 0, 2)
            t_h1 = sb.tile([NN, 1], f32)
            nc.vector.tensor_scalar(t_h1[:], t_gi[:], -1.0, 1.0, OP.mult,
                                    OP.add)
            nc.vector.tensor_scalar(t_R[0:NN, 0:1], t_h1[:], 0.0, 2.0,
                                    OP.max, OP.min)
            nc.vector.tensor_tensor(t_R[0:NN, 2:3], t_co[:], t_va, OP.mult)
            ps_out = ps.tile([4, 4], f32)
            nc.tensor.matmul(ps_out[:], t_R[:], t_indD[:])
            t_os = sb.tile([4, 4], f32)
            nc.vector.tensor_copy(t_os[:], ps_out[:])
            nc.sync.dma_start(osum[:], t_os[:])

    nc.finalize()
    for blk in nc.m.functions[0].blocks:
        for ins in blk.instructions:
            si = ins.sync_info
            nw = len(si.on_wait) if (si and si.on_wait) else 0
            cap = 2 if type(ins).__name__ == "InstDMACopy" else 1
            if nw > cap:
                import os as _os
                if _os.environ.get("BASSDL_NO_WAIT_ASSERT"):
                    print("WAITVIOLATION", type(ins).__name__, ins.name,
                          ins.engine, [x.ant_name for x in si.on_wait])
                else:
                    raise AssertionError(
                        f"{type(ins).__name__} {ins.name} has {nw} sync waits "
                        f"(cap {cap} in this walrus build) — restructure deps")
    return nc


def host_prep(objectness, boxes, classes, gt_boxes, gt_labels):
    """Build the 8 per-core input maps.  Index/one-hot prep from gt_* plus
    pure layout transforms (transposes) of the float inputs — no float
    loss math happens here."""
    objectness = np.ascontiguousarray(np.asarray(objectness, dtype=np.float32))
    boxes = np.asarray(boxes, dtype=np.float32)
    classes = np.asarray(classes, dtype=np.float32)
    gb = np.asarray(gt_boxes, dtype=np.float32)
    gl = np.asarray(gt_labels).astype(np.int64)

    cx = np.clip((gb[:, :, 0] * np.float32(W)).astype(np.int32), 0, W - 1)
    cy = np.clip((gb[:, :, 1] * np.float32(H)).astype(np.int32), 0, H - 1)
    s = (cy * W + cx).astype(np.int64)                      # [B,N]
    eq = s[:, :, None] == s[:, None, :]                     # [B,N,N]
    tril = np.tril(np.ones((N, N), dtype=bool), k=-1)
    rank = (eq & tril[None]).sum(axis=2)                    # [B,N]
    valid = rank < M
    slot_t = np.minimum(rank, M - 1)

    # cold params
    cold = np.zeros((128, COLD), np.float32)
    for i in range(BC):
        cold[N * i:N * (i + 1), 64 + i] = 1.0               # ind20
        cold[64 * i:64 * (i + 1), 66 + i] = -1.0            # ind_neg

    in_maps = []
    for c in range(NCORES):
        bsel = slice(BC * c, BC * (c + 1))
        sB = s[bsel]                                        # [BC,N]
        il = np.arange(BC, dtype=np.int64)[:, None]
        pidx = (il * HW + sB).reshape(NN).astype(np.int32)

        glc = gl[bsel].reshape(NN)
        ohc = np.zeros((NN, C), np.float32)
        ohc[np.arange(NN), glc] = 1.0
        al = np.where(glc == 0, np.float32(ALPHA), np.float32(1 - ALPHA))
        va = valid[bsel].reshape(NN).astype(np.float32)
        oht = np.zeros((NN, M), np.float32)
        oht[np.arange(NN), slot_t[bsel].reshape(NN)] = 1.0

        hot = np.zeros((NN, HOT), np.float32)
        hot[:, 0] = pidx.view(np.float32)
        hot[:, 1:5] = gb[bsel].reshape(NN, 4)
        hot[:, 5:13] = oht
        hot[:, 13] = va
        hot[:, 14] = al
        hot[:, 15:23] = (np.arange(M) + 1000.0).astype(np.float32)[None, :]
        hot[:, 23:31] = np.arange(M, dtype=np.float32)[None, :]

        coldc = cold.copy()
        coldc[0:NN, 0:64] = ohc

        pk = np.empty((BC, HW, PK), np.float32)
        pk[:, :, 0:8] = objectness[bsel].transpose(0, 2, 3, 1).reshape(
            BC, HW, M)
        pk[:, :, 8:40] = boxes[bsel].transpose(0, 3, 4, 1, 2).reshape(
            BC, HW, M * 4)
        pk[:, :, 40:PK] = classes[bsel].transpose(0, 3, 4, 2, 1).reshape(
            BC, HW, C * M)

        in_maps.append({
            "obj": objectness[bsel].reshape(-1),
            "pack": pk.reshape(-1),
            "ph": hot,
            "pc": coldc,
        })
    return in_maps


def assemble(results):
    """Unshard: per-core [4,4] sums -> three weighted scalar means."""
    box, cls_, objl = [], [], []
    for r in results:
        o = np.asarray(r["osum"], dtype=np.float32)
        for i in range(BC):
            box.append(o[0, i] / np.float32(N))
            cls_.append(o[1, i] / np.float32(N))
            objl.append((o[2, i] + o[3, 2 + i]) / np.float32(M * HW))
    bl = np.float32(np.sum(np.asarray(box, np.float32)) / np.float32(B))
    cl = np.float32(np.sum(np.asarray(cls_, np.float32)) / np.float32(B))
    ol = np.float32(np.sum(np.asarray(objl, np.float32)) / np.float32(B))
    return (np.float32(bl * np.float32(BOX_W)),
            np.float32(cl * np.float32(CLS_W)),
            np.float32(ol * np.float32(OBJ_W)))


def _get_program():
    global _PROG
    if _PROG is None:
        _PROG = build_program()
    return _PROG


LAST_RESULTS = None  # BassKernelResults of the most recent run (for test.py)


def kernel(objectness, boxes, classes, gt_boxes, gt_labels):
    import os
    from concourse.bass_utils import run_bass_kernel_spmd

    global LAST_RESULTS
    nc = _get_program()
    in_maps = host_prep(objectness, boxes, classes, gt_boxes, gt_labels)
    trace = bool(os.environ.get("BASSDL_TRACE"))
    res = run_bass_kernel_spmd(nc, in_maps, list(range(NCORES)), trace=trace)
    LAST_RESULTS = res
    return assemble(res.results)
